# revision 24
# baseline (speedup 1.0000x reference)
"""Trainium2 Bass kernel for a fused MultiHead-GAT layer (8-core SPMD).

Strategy (edges sharded by balanced dst blocks; tgt nodes data-parallel):
  host:  assign dst nodes to 80 edge-balanced blocks of <=128 nodes
         (10 blocks/core, CPB uniform), group edges by block, pre-transpose
         edge_embed chunks, pre-build one-hot (edge->local slot) chunks,
         fold attn_a into tiny weight matrices; final output rows are
         un-permuted on host.
  device (per core):
    z rows  = src_h_shard @ W_fc (columns permuted o*8+h), s1 = src_h @ (W_fc @ Ablk)
    AllGather -> full [N_SRC, 640] bf16 table (z|s1|pad)
    gathers run as SWDGE prepare_only descriptor-gen (hoisted into the
    preamble / overlapped with compute on GpSimd) + per-block trigger_dma;
    per block: s2 via PE (edge_embed^T @ V), e = leaky(s1+s2) on Scalar;
    softmax without max-shift; eexp weighting multiplied IN PLACE into the
    gathered z tile; aggregation via one-hot matmul into PSUM [t, 512+8];
    divide+unpermute, elu+residual (Scalar Relu/Exp + min(exp,1) trick);
    FFN (bf16 matmuls) feature-major, interleaved with the block loop;
    PE-transpose to node-major; LayerNorm per node (Scalar accum_out for
    the reductions); f32 output.
"""
import sys

sys.path.insert(0, "/opt/trn_rl_repo")

from contextlib import ExitStack
from types import SimpleNamespace

import numpy as np
import ml_dtypes

import concourse.bass as bass
import concourse.bacc as bacc
import concourse.tile as tile
from concourse import mybir

BF16 = mybir.dt.bfloat16
F32 = mybir.dt.float32
I16 = mybir.dt.int16
NP_BF16 = ml_dtypes.bfloat16

LN_EPS = 1e-5
LEAK = 0.01


def full_cfg():
    return SimpleNamespace(
        ncores=8,
        n_src=10000, n_tgt=10000, e=160000,
        in_dim=512, d=512, h=8, o=64, ed=128, fh=2048,
        tgt_per=1250, tgt_pad=1280, nblk=10,
        zrow=640,  # 512 z + 8 s1 + 120 pad (row bytes % 256 == 0)
    )


def _balance_blocks(deg, nblocks, slots_per_block):
    """Assign nodes to blocks, balancing total degree, <=slots nodes/block."""
    import heapq
    order = np.argsort(-deg, kind="stable")
    heap = [(0, b) for b in range(nblocks)]  # (edges, block)
    heapq.heapify(heap)
    nslots = np.zeros(nblocks, np.int64)
    block_of = np.empty(len(deg), np.int64)
    slot_of = np.empty(len(deg), np.int64)
    spill = []
    for n in order:
        while True:
            e_cnt, b = heapq.heappop(heap)
            if nslots[b] < slots_per_block:
                break
            spill.append((e_cnt, b))
        block_of[n] = b
        slot_of[n] = nslots[b]
        nslots[b] += 1
        heapq.heappush(heap, (e_cnt + int(deg[n]), b))
        while spill:
            heapq.heappush(heap, spill.pop())
    return block_of, slot_of


def host_prep(cfg, src_h, tgt_h, edge_embed, edge_src, edge_dst,
              W_fc, W_feat, attn_a, w1, b1, w2, b2, ln_g, ln_b):
    C = cfg
    H, O, D = C.h, C.o, C.d
    NBLOCKS = C.ncores * C.nblk

    deg = np.bincount(np.asarray(edge_dst), minlength=C.n_tgt)
    block_of, slot_of = _balance_blocks(deg, NBLOCKS, 128)

    eb_blk = block_of[np.asarray(edge_dst)]
    perm = np.argsort(eb_blk, kind="stable")
    es = np.asarray(edge_src)[perm].astype(np.int64)
    ed_blk = eb_blk[perm]
    lt_all = slot_of[np.asarray(edge_dst)][perm]
    ee = np.asarray(edge_embed)[perm]
    bstart = np.searchsorted(ed_blk, np.arange(NBLOCKS + 1))

    cnts = bstart[1:] - bstart[:-1]
    cpb = max(1, int((cnts.max() + 127) // 128))
    C.cpb = cpb

    # feature permutation q = o*8+h  <->  f = h*64+o
    q = np.arange(D)
    f_of_q = (q % H) * O + (q // H)           # column f placed at position q
    Wfc_p = np.asarray(W_fc)[:, f_of_q]       # z_perm = src_h @ Wfc_p

    a_src = np.asarray(attn_a)[0, :, :O]       # [H, O]
    a_feat = np.asarray(attn_a)[0, :, 2 * O:]  # [H, O]
    Ablk = np.zeros((D, H), np.float32)
    for h in range(H):
        Ablk[h * O:(h + 1) * O, h] = a_src[h]
    M1 = (np.asarray(W_fc, np.float64) @ Ablk.astype(np.float64)).astype(np.float32)
    V = np.zeros((C.ed, H), np.float32)
    for h in range(H):
        V[:, h] = np.asarray(W_feat)[:, h * O:(h + 1) * O] @ a_feat[h]

    cores = []
    for c in range(C.ncores):
        idxw = np.zeros((C.nblk, 128, cpb * 8), np.int16)
        eeT = np.zeros((C.nblk, cpb, 128, 128), NP_BF16)
        oh = np.zeros((C.nblk, cpb, 128, 128), NP_BF16)
        th = np.zeros((C.tgt_pad, D), np.float32)
        for b in range(C.nblk):
            g = c * C.nblk + b
            s, t = bstart[g], bstart[g + 1]
            n = t - s
            if n > 0:
                src_b = np.zeros(cpb * 128, np.int64)
                src_b[:n] = es[s:t]
                lt = np.full(cpb * 128, -1, np.int64)
                lt[:n] = lt_all[s:t]
                # gather index wrap: logical i -> partition i%16, col i//16, x8
                base = src_b.astype(np.int16).reshape(-1, 16).T  # [16, cpb*8]
                for k in range(8):
                    idxw[b, k * 16:(k + 1) * 16, :] = base
                eb = np.zeros((cpb * 128, C.ed), NP_BF16)
                eb[:n] = ee[s:t].astype(NP_BF16)
                eeT[b] = eb.reshape(cpb, 128, C.ed).transpose(0, 2, 1)
                ohb = np.zeros((cpb * 128, 128), NP_BF16)
                valid = lt >= 0
                ohb[np.nonzero(valid)[0], lt[valid]] = 1.0
                oh[b] = ohb.reshape(cpb, 128, 128)
            # tgt_h rows for this block's slots (elu trick: + (tgt_h - 1))
            nodes = np.nonzero(block_of == g)[0]
            th[b * 128 + slot_of[nodes]] = np.asarray(tgt_h)[nodes] - 1.0

        sh = np.zeros((C.in_dim, C.tgt_pad), np.float32)
        lo2 = c * C.tgt_per
        hi2 = min((c + 1) * C.tgt_per, C.n_src)
        sh[:, :hi2 - lo2] = np.asarray(src_h)[lo2:hi2].T

        cores.append({
            "idxw": idxw, "eeT": eeT, "oh": oh,
            "tgt_hm1": th.astype(NP_BF16),
            "src_hT": sh.astype(NP_BF16),
        })

    # inverse permutation: node -> row in concatenated shard outputs
    row_of_node = (block_of // C.nblk) * C.tgt_pad + \
                  (block_of % C.nblk) * 128 + slot_of

    shared = {
        "wfc": Wfc_p.astype(NP_BF16),
        "m1": M1.astype(NP_BF16),
        "v": V.astype(NP_BF16),
        "w1": np.asarray(w1).astype(NP_BF16),
        "w2": np.asarray(w2).astype(NP_BF16),
        "b1c": np.asarray(b1, np.float32).reshape(C.fh, 1),
        "b2c": np.asarray(b2, np.float32).reshape(D, 1),
        "g_rep": np.tile(np.asarray(ln_g, np.float32).reshape(1, D), (128, 1)),
        "b_rep": np.tile(np.asarray(ln_b, np.float32).reshape(1, D), (128, 1)),
        "identb": np.eye(128, dtype=NP_BF16),
    }
    return cores, shared, row_of_node


def build_program(C):
    nc = bacc.Bacc("TRN2", target_bir_lowering=False, debug=False,
                   num_devices=C.ncores)
    H, O, D, CPB, NBLK = C.h, C.o, C.d, C.cpb, C.nblk
    ZR = C.zrow
    NPAD = C.tgt_pad
    BUFS = 4            # zg lookahead depth
    GCALL = 4           # gather chunks per swdge call (512 idxs)

    # -------- I/O --------
    def din(name, shape, dt):
        return nc.dram_tensor(name, shape, dt, kind="ExternalInput").ap()

    idxw = din("idxw", [NBLK, 128, CPB * 8], I16)
    eeT = din("eeT", [NBLK, CPB, 128, 128], BF16)
    oh = din("oh", [NBLK, CPB, 128, 128], BF16)
    tgt_hm1 = din("tgt_hm1", [NPAD, D], BF16)
    src_hT = din("src_hT", [C.in_dim, NPAD], BF16)
    wfc = din("wfc", [C.in_dim, D], BF16)
    m1 = din("m1", [C.in_dim, H], BF16)
    vmat = din("v", [C.ed, H], BF16)
    w1 = din("w1", [D, C.fh], BF16)
    w2 = din("w2", [C.fh, D], BF16)
    b1c = din("b1c", [C.fh, 1], F32)
    b2c = din("b2c", [D, 1], F32)
    g_rep = din("g_rep", [128, D], F32)
    b_rep = din("b_rep", [128, D], F32)
    identb = din("identb", [128, 128], BF16)

    out_shard = nc.dram_tensor("out_shard", [NPAD, D], F32,
                               kind="ExternalOutput").ap()

    zc_bounce = nc.dram_tensor("zc_bounce", [C.tgt_per, ZR], BF16).ap()
    zc_space = "Shared" if C.ncores > 4 else None
    zc_table = nc.dram_tensor("zc_table", [C.n_src, ZR], BF16,
                              addr_space=zc_space).ap()

    KT = C.in_dim // 128   # 4
    FT = D // 128          # 4
    MT1 = C.fh // 128      # 16
    chunks = [(0, 256), (256, 256), (512, 512), (1024, 256)]

    with tile.TileContext(nc) as tc, ExitStack() as top:
        const = top.enter_context(tc.tile_pool(name="const", bufs=1))
        zgp = top.enter_context(tc.tile_pool(name="zg", bufs=BUFS))

        # idx table first: SWDGE preps consume it
        idx_sb = const.tile([128, NBLK, CPB * 8], I16)
        nc.sync.dma_start(idx_sb[:], idxw[:, :, :].rearrange("b p s -> p b s"))
        v_sb = const.tile([128, H], BF16)
        nc.sync.dma_start(v_sb[:], vmat[:, :])
        idb_sb = const.tile([128, 128], BF16)
        nc.sync.dma_start(idb_sb[:], identb[:, :])
        zero_sb = const.tile([128, 1], F32)
        nc.vector.memset(zero_sb[:], 0.0)
        eps_sb = const.tile([128, 1], F32)
        nc.vector.memset(eps_sb[:], LN_EPS)
        zpad_sb = const.tile([128, ZR - D - H], BF16)
        nc.vector.memset(zpad_sb[:], 0.0)
        # (w1/w2/ln consts DMA'd after phase 0 below; allocated now)
        w1_sb = const.tile([128, KT, MT1, 128], BF16)
        w2_sb = const.tile([128, MT1, FT, 128], BF16)
        b1_sb = const.tile([128, MT1, 1], F32)
        b2_sb = const.tile([128, FT, 1], F32)
        grep_sb = const.tile([128, D], F32)
        brep_sb = const.tile([128, D], F32)

        zg_tiles = {}

        def emit_gather(nb):
            """Gather issued BUFS blocks ahead of consumption so GpSimd
            desc-gen overlaps compute on the other engines."""
            zg = zgp.tile([128, CPB, ZR], BF16, tag="zg")
            zg_tiles[nb] = zg
            for g0 in range(0, CPB, GCALL):
                gn = min(GCALL, CPB - g0)
                nc.gpsimd.dma_gather(
                    out_ap=zg[:, g0:g0 + gn, :], in_ap=zc_table[:, :],
                    idxs_ap=idx_sb[:, nb, g0 * 8:(g0 + gn) * 8],
                    num_idxs=gn * 128, num_idxs_reg=gn * 128, elem_size=ZR)

        # ---------------- phase 0: z rows + s1 rows -> zc_bounce -> AllGather
        with ExitStack() as p0:
            ps0 = p0.enter_context(tc.tile_pool(name="ps0", bufs=2, space="PSUM"))
            zr_pool = p0.enter_context(tc.tile_pool(name="zrow", bufs=2))
            shp = p0.enter_context(tc.tile_pool(name="shp", bufs=1))
            sh_sb = shp.tile([128, KT, NPAD], BF16)
            nc.sync.dma_start(sh_sb[:],
                              src_hT.rearrange("(kt p) n -> p kt n", p=128))
            wfc_sb = shp.tile([128, KT, D], BF16)
            nc.sync.dma_start(wfc_sb[:], wfc.rearrange("(kt p) m -> p kt m", p=128))
            m1_sb = shp.tile([128, KT, H], BF16)
            nc.sync.dma_start(m1_sb[:], m1.rearrange("(kt p) m -> p kt m", p=128))
            for nb in range(NBLK):
                rows = min(128, C.tgt_per - nb * 128)
                if rows <= 0:
                    break
                z_ps = ps0.tile([128, D], F32, tag="zps")
                for kt in range(KT):
                    nc.tensor.matmul(z_ps[:], sh_sb[:, kt, nb * 128:(nb + 1) * 128],
                                     wfc_sb[:, kt, :], start=(kt == 0),
                                     stop=(kt == KT - 1))
                s1_ps = ps0.tile([128, H], F32, tag="s1ps")
                for kt in range(KT):
                    nc.tensor.matmul(s1_ps[:], sh_sb[:, kt, nb * 128:(nb + 1) * 128],
                                     m1_sb[:, kt, :], start=(kt == 0),
                                     stop=(kt == KT - 1))
                zrow = zr_pool.tile([128, D], BF16, tag="zrow")
                nc.vector.tensor_copy(zrow[:], z_ps[:])
                s1row = zr_pool.tile([128, H], BF16, tag="s1row")
                nc.vector.tensor_copy(s1row[:], s1_ps[:])
                nc.sync.dma_start(zc_bounce[nb * 128:nb * 128 + rows, 0:D],
                                  zrow[0:rows, :])
                nc.sync.dma_start(zc_bounce[nb * 128:nb * 128 + rows, D:D + H],
                                  s1row[0:rows, :])
                nc.sync.dma_start(zc_bounce[nb * 128:nb * 128 + rows, D + H:ZR],
                                  zpad_sb[0:rows, :])

        nc.gpsimd.collective_compute(
            "AllGather", mybir.AluOpType.bypass,
            replica_groups=[list(range(C.ncores))],
            ins=[zc_bounce[:, :]], outs=[zc_table[:, :]],
        )

        for nb0 in range(BUFS):
            emit_gather(nb0)

        # FFN / LN constants (not needed until the first FFN chunk)
        nc.sync.dma_start(w1_sb[:], w1.rearrange("(kt p) (mt m) -> p kt mt m",
                                                 p=128, m=128))
        nc.sync.dma_start(w2_sb[:], w2.rearrange("(kt p) (mt m) -> p kt mt m",
                                                 p=128, m=128))
        nc.sync.dma_start(b1_sb[:], b1c.rearrange("(mt p) x -> p mt x", p=128))
        nc.sync.dma_start(b2_sb[:], b2c.rearrange("(mt p) x -> p mt x", p=128))
        nc.sync.dma_start(grep_sb[:], g_rep[:, :])
        nc.sync.dma_start(brep_sb[:], b_rep[:, :])

        # ---------------- blocks + FFN, interleaved so PE fills gather gaps
        with ExitStack() as pb:
            ps_s2 = pb.enter_context(tc.tile_pool(name="ps_s2", bufs=1, space="PSUM"))
            ps_hag = pb.enter_context(tc.tile_pool(name="ps_hag", bufs=1, space="PSUM"))
            ps_tp = pb.enter_context(tc.tile_pool(name="ps_tp", bufs=1, space="PSUM"))
            ps_a1 = pb.enter_context(tc.tile_pool(name="ps_a1", bufs=2, space="PSUM"))
            ps_o2 = pb.enter_context(tc.tile_pool(name="ps_o2", bufs=2, space="PSUM"))
            gpool = pb.enter_context(tc.tile_pool(name="ee_oh", bufs=2))
            epool = pb.enter_context(tc.tile_pool(name="escore", bufs=2))
            hpool = pb.enter_context(tc.tile_pool(name="hb", bufs=1))
            tgtp = pb.enter_context(tc.tile_pool(name="tgtp", bufs=2))
            hbtp = pb.enter_context(tc.tile_pool(name="hbt", bufs=1))
            fpool = pb.enter_context(tc.tile_pool(name="ffn", bufs=1))
            r1p = pb.enter_context(tc.tile_pool(name="r1", bufs=1))
            tmpp = pb.enter_context(tc.tile_pool(name="tmp", bufs=2))
            lnp = pb.enter_context(tc.tile_pool(name="ln", bufs=1))
            stp = pb.enter_context(tc.tile_pool(name="stat", bufs=2))

            hbT = hbtp.tile([128, FT, NPAD], BF16)
            r2 = fpool.tile([128, FT, NPAD], BF16)

            def emit_ffn_chunk(cs, cw):
                r1 = r1p.tile([128, MT1, cw], BF16, tag="r1")
                for mt in range(MT1):
                    a1 = ps_a1.tile([128, cw], F32, tag="a1")
                    for kt in range(KT):
                        nc.tensor.matmul(a1[:], w1_sb[:, kt, mt, :],
                                         hbT[:, kt, cs:cs + cw],
                                         start=(kt == 0), stop=(kt == KT - 1))
                    if mt % 2 == 0:
                        nc.scalar.activation(r1[:, mt, :], a1[:],
                                             mybir.ActivationFunctionType.Relu,
                                             bias=b1_sb[:, mt, :])
                    else:
                        nc.vector.tensor_scalar(r1[:, mt, :], a1[:],
                                                b1_sb[:, mt, :], 0.0,
                                                mybir.AluOpType.add,
                                                mybir.AluOpType.max)
                for mt2 in range(FT):
                    o2 = ps_o2.tile([128, cw], F32, tag="o2")
                    for kt2 in range(MT1):
                        nc.tensor.matmul(o2[:], w2_sb[:, kt2, mt2, :],
                                         r1[:, kt2, :],
                                         start=(kt2 == 0), stop=(kt2 == MT1 - 1))
                    t1 = tmpp.tile([128, cw], F32, tag="t1")
                    nc.vector.tensor_scalar_add(t1[:], o2[:], b2_sb[:, mt2, :])
                    nc.vector.tensor_tensor(r2[:, mt2, cs:cs + cw], t1[:],
                                            hbT[:, mt2, cs:cs + cw],
                                            mybir.AluOpType.add)

            def emit_ln(nb):
                r2n = lnp.tile([128, D], BF16, tag="r2n")
                for ft in range(FT):
                    tp = ps_tp.tile([128, 128], BF16, tag="tp")
                    nc.tensor.transpose(tp[:], r2[:, ft, nb * 128:(nb + 1) * 128],
                                        idb_sb[:])
                    nc.vector.tensor_copy(r2n[:, ft * 128:(ft + 1) * 128], tp[:])
                scrap = lnp.tile([128, D], BF16, tag="scrap")
                ssum = stp.tile([128, 1], F32, tag="ssum")
                nc.scalar.activation(scrap[:], r2n[:],
                                     mybir.ActivationFunctionType.Copy,
                                     accum_out=ssum[:])
                mu = stp.tile([128, 1], F32, tag="mu")
                nc.vector.tensor_scalar_mul(mu[:], ssum[:], 1.0 / D)
                xc = lnp.tile([128, D], BF16, tag="xc")
                nc.vector.tensor_scalar(xc[:], r2n[:], mu[:], None,
                                        mybir.AluOpType.subtract)
                sq = lnp.tile([128, D], BF16, tag="sq")
                ssq = stp.tile([128, 1], F32, tag="ssq")
                nc.scalar.activation(sq[:], xc[:],
                                     mybir.ActivationFunctionType.Square,
                                     accum_out=ssq[:])
                std = stp.tile([128, 1], F32, tag="std")
                nc.scalar.activation(std[:], ssq[:],
                                     mybir.ActivationFunctionType.Sqrt,
                                     bias=eps_sb[:, :], scale=1.0 / D)
                rstd = stp.tile([128, 1], F32, tag="rstd")
                nc.vector.reciprocal(rstd[:], std[:])
                xn = lnp.tile([128, D], BF16, tag="xn")
                nc.vector.tensor_scalar_mul(xn[:], xc[:], rstd[:])
                xg = lnp.tile([128, D], F32, tag="xg")
                nc.vector.tensor_tensor(xg[:], xn[:], grep_sb[:],
                                        mybir.AluOpType.mult)
                orow = lnp.tile([128, D], F32, tag="orow")
                nc.vector.tensor_tensor(orow[:], xg[:], brep_sb[:],
                                        mybir.AluOpType.add)
                nc.sync.dma_start(out_shard[nb * 128:(nb + 1) * 128, :], orow[:])

            next_chunk = 0
            for nb in range(NBLK):
                zg = zg_tiles[nb]
                ee_t = gpool.tile([128, CPB, 128], BF16, tag="ee")
                nc.sync.dma_start(ee_t[:], eeT[nb].rearrange("c d e -> d c e"))
                oh_t = gpool.tile([128, CPB, 128], BF16, tag="oh")
                nc.sync.dma_start(oh_t[:], oh[nb].rearrange("c e t -> e c t"))
                tgtb = tgtp.tile([128, D], BF16, tag="tgtb")
                nc.sync.dma_start(tgtb[:], tgt_hm1[nb * 128:(nb + 1) * 128, :])

                s2_ps = ps_s2.tile([128, CPB * H], F32, tag="s2")
                for j in range(CPB):
                    nc.tensor.matmul(s2_ps[:, j * H:(j + 1) * H], ee_t[:, j, :],
                                     v_sb[:, :], start=True, stop=True)
                e1 = epool.tile([128, CPB, H], F32, tag="e1")
                nc.vector.tensor_tensor(
                    e1[:, :, :],
                    s2_ps[:, :].rearrange("p (c h) -> p c h", h=H),
                    zg[:, :, D:D + H],
                    mybir.AluOpType.add)
                lk = epool.tile([128, CPB, H], F32, tag="lk")
                nc.vector.tensor_scalar_mul(lk[:, :, :], e1[:, :, :], LEAK)
                e2 = epool.tile([128, CPB, H], F32, tag="e2")
                nc.vector.tensor_tensor(e2[:, :, :], e1[:, :, :], lk[:, :, :],
                                        mybir.AluOpType.max)
                eexp = epool.tile([128, CPB, H], BF16, tag="eexp")
                nc.scalar.activation(eexp[:, :, :], e2[:, :, :],
                                     mybir.ActivationFunctionType.Exp,
                                     bias=zero_sb[:, :])

                # eexp weighting in place into the gathered z columns;
                # split across Vector and GpSimd to unload the DVE
                CSPL = (CPB * 5) // 8
                nc.vector.tensor_tensor(
                    zg[:, 0:CSPL, 0:D].rearrange("p c (o h) -> p c o h", h=H),
                    zg[:, 0:CSPL, 0:D].rearrange("p c (o h) -> p c o h", h=H),
                    eexp[:, 0:CSPL, :].rearrange("p c (h x) -> p c x h", x=1)
                        .broadcast_to([128, CSPL, O, H]),
                    mybir.AluOpType.mult)
                nc.gpsimd.tensor_tensor(
                    zg[:, CSPL:CPB, 0:D].rearrange("p c (o h) -> p c o h", h=H),
                    zg[:, CSPL:CPB, 0:D].rearrange("p c (o h) -> p c o h", h=H),
                    eexp[:, CSPL:CPB, :].rearrange("p c (h x) -> p c x h", x=1)
                        .broadcast_to([128, CPB - CSPL, O, H]),
                    mybir.AluOpType.mult)

                hag = ps_hag.tile([128, D + H], F32, tag="hag")
                for j in range(CPB):
                    nc.tensor.matmul(hag[:, 0:D], oh_t[:, j, :], zg[:, j, 0:D],
                                     start=(j == 0), stop=(j == CPB - 1),
                                     skip_group_check=True)
                    nc.tensor.matmul(hag[:, D:D + H], oh_t[:, j, :], eexp[:, j, :],
                                     start=(j == 0), stop=(j == CPB - 1),
                                     skip_group_check=True)

                den = epool.tile([128, H], F32, tag="den")
                nc.vector.tensor_scalar_max(den[:], hag[:, D:D + H], 1e-30)
                rec = epool.tile([128, H], F32, tag="rec")
                nc.vector.reciprocal(rec[:], den[:])

                hbp = hpool.tile([128, D], BF16, tag="hbp")
                nc.vector.tensor_tensor(
                    hbp[:, :].rearrange("p (h o) -> p h o", o=O),
                    hag[:, 0:D].rearrange("p (o h) -> p h o", h=H),
                    rec[:, :].rearrange("p (h x) -> p h x", x=1)
                        .broadcast_to([128, H, O]),
                    mybir.AluOpType.mult)
                # elu(x) + tgt = max(x,0) + min(exp(x),1) + (tgt-1)
                mx = hpool.tile([128, D], BF16, tag="mx")
                nc.scalar.activation(mx[:], hbp[:],
                                     mybir.ActivationFunctionType.Relu)
                ex = hpool.tile([128, D], BF16, tag="ex")
                nc.scalar.activation(ex[:], hbp[:],
                                     mybir.ActivationFunctionType.Exp,
                                     bias=zero_sb[:, :])
                ex1 = hpool.tile([128, D], BF16, tag="ex1")
                nc.vector.tensor_scalar_min(ex1[:], ex[:], 1.0)
                hb2 = hpool.tile([128, D], BF16, tag="hb2")
                nc.vector.tensor_tensor(hb2[:], ex1[:], mx[:], mybir.AluOpType.add)
                hb3 = hpool.tile([128, D], BF16, tag="hb3")
                nc.vector.tensor_tensor(hb3[:], hb2[:], tgtb[:],
                                        mybir.AluOpType.add)
                for ft in range(FT):
                    tpb = ps_tp.tile([128, 128], BF16, tag="tp")
                    nc.tensor.transpose(tpb[:], hb3[:, ft * 128:(ft + 1) * 128],
                                        idb_sb[:])
                    nc.vector.tensor_copy(hbT[:, ft, nb * 128:(nb + 1) * 128],
                                          tpb[:])

                # prep+fire gather nb+BUFS (its zg slot was freed by block nb)
                if nb + BUFS < NBLK:
                    emit_gather(nb + BUFS)

                while (next_chunk < len(chunks)
                       and chunks[next_chunk][0] + chunks[next_chunk][1]
                       <= (nb + 1) * 128):
                    cs, cw = chunks[next_chunk]
                    emit_ffn_chunk(cs, cw)
                    for nb_ln in range(cs // 128, (cs + cw) // 128):
                        emit_ln(nb_ln)
                    next_chunk += 1

    nc.compile()
    return nc


_CACHE = {}


def _get_program(C):
    key = (C.ncores, C.n_src, C.n_tgt, C.e, C.cpb)
    if key not in _CACHE:
        _CACHE[key] = build_program(C)
    return _CACHE[key]


def kernel(src_h, tgt_h, edge_embed, edge_src, edge_dst,
           W_fc, W_feat, attn_a, w1, b1, w2, b2, ln_g, ln_b):
    from concourse.bass_utils import run_bass_kernel_spmd

    C = full_cfg()
    cores, shared, row_of_node = host_prep(
        C, src_h, tgt_h, edge_embed, edge_src, edge_dst,
        W_fc, W_feat, attn_a, w1, b1, w2, b2, ln_g, ln_b)
    nc = _get_program(C)
    in_maps = []
    for c in range(C.ncores):
        m = dict(shared)
        cc = cores[c]
        m.update(idxw=cc["idxw"], eeT=cc["eeT"], oh=cc["oh"],
                 tgt_hm1=cc["tgt_hm1"], src_hT=cc["src_hT"])
        in_maps.append(m)
    import os
    try:
        res = run_bass_kernel_spmd(nc, in_maps, list(range(C.ncores)))
    except Exception:
        if os.environ.get("BASS_TRACE"):
            os.environ["BASS_NEVER_TRACE"] = "1"
            res = run_bass_kernel_spmd(nc, in_maps, list(range(C.ncores)))
        else:
            raise
    global _last_results
    _last_results = res
    allrows = np.concatenate(
        [res.results[c]["out_shard"] for c in range(C.ncores)], axis=0)
    out = allrows[row_of_node]
    return np.ascontiguousarray(out, dtype=np.float32)


# revision 26
# speedup vs baseline: 1.2574x; 1.2574x over previous
"""Trainium2 Bass kernel for a fused MultiHead-GAT layer (8-core SPMD).

Strategy (edges sharded by balanced dst blocks; tgt nodes data-parallel):
  host:  assign dst nodes to 80 edge-balanced blocks of <=128 nodes
         (10 blocks/core, CPB uniform), group edges by block, pre-transpose
         edge_embed chunks, pre-build one-hot (edge->local slot) chunks,
         fold attn_a into tiny weight matrices; final output rows are
         un-permuted on host.
  device (per core):
    z rows  = src_h_shard @ W_fc (columns permuted o*8+h), s1 = src_h @ (W_fc @ Ablk)
    AllGather -> full [N_SRC, 640] bf16 table (z|s1|pad)
    gathers run as SWDGE prepare_only descriptor-gen (hoisted into the
    preamble / overlapped with compute on GpSimd) + per-block trigger_dma;
    per block: s2 via PE (edge_embed^T @ V), e = leaky(s1+s2) on Scalar;
    softmax without max-shift; eexp weighting multiplied IN PLACE into the
    gathered z tile; aggregation via one-hot matmul into PSUM [t, 512+8];
    divide+unpermute, elu+residual (Scalar Relu/Exp + min(exp,1) trick);
    FFN (bf16 matmuls) feature-major, interleaved with the block loop;
    PE-transpose to node-major; LayerNorm per node (Scalar accum_out for
    the reductions); f32 output.
"""
import sys

sys.path.insert(0, "/opt/trn_rl_repo")

from contextlib import ExitStack
from types import SimpleNamespace

import numpy as np
import ml_dtypes

import concourse.bass as bass
import concourse.bacc as bacc
import concourse.tile as tile
from concourse import mybir

BF16 = mybir.dt.bfloat16
F32 = mybir.dt.float32
I16 = mybir.dt.int16
NP_BF16 = ml_dtypes.bfloat16

LN_EPS = 1e-5
LEAK = 0.01


def full_cfg():
    return SimpleNamespace(
        ncores=8,
        n_src=10000, n_tgt=10000, e=160000,
        in_dim=512, d=512, h=8, o=64, ed=128, fh=2048,
        tgt_per=1250, tgt_pad=1280, nblk=10,
        zrow=640,  # 512 z + 8 s1 + 120 pad (row bytes % 256 == 0)
    )


def _balance_blocks(deg, nblocks, slots_per_block):
    """Assign nodes to blocks, balancing total degree, <=slots nodes/block."""
    import heapq
    order = np.argsort(-deg, kind="stable")
    heap = [(0, b) for b in range(nblocks)]  # (edges, block)
    heapq.heapify(heap)
    nslots = np.zeros(nblocks, np.int64)
    block_of = np.empty(len(deg), np.int64)
    slot_of = np.empty(len(deg), np.int64)
    spill = []
    for n in order:
        while True:
            e_cnt, b = heapq.heappop(heap)
            if nslots[b] < slots_per_block:
                break
            spill.append((e_cnt, b))
        block_of[n] = b
        slot_of[n] = nslots[b]
        nslots[b] += 1
        heapq.heappush(heap, (e_cnt + int(deg[n]), b))
        while spill:
            heapq.heappush(heap, spill.pop())
    return block_of, slot_of


def host_prep(cfg, src_h, tgt_h, edge_embed, edge_src, edge_dst,
              W_fc, W_feat, attn_a, w1, b1, w2, b2, ln_g, ln_b):
    C = cfg
    H, O, D = C.h, C.o, C.d
    NBLOCKS = C.ncores * C.nblk

    deg = np.bincount(np.asarray(edge_dst), minlength=C.n_tgt)
    block_of, slot_of = _balance_blocks(deg, NBLOCKS, 128)

    eb_blk = block_of[np.asarray(edge_dst)]
    perm = np.argsort(eb_blk, kind="stable")
    es = np.asarray(edge_src)[perm].astype(np.int64)
    ed_blk = eb_blk[perm]
    lt_all = slot_of[np.asarray(edge_dst)][perm]
    ee = np.asarray(edge_embed)[perm]
    bstart = np.searchsorted(ed_blk, np.arange(NBLOCKS + 1))

    cnts = bstart[1:] - bstart[:-1]
    cpb = max(1, int((cnts.max() + 127) // 128))
    C.cpb = cpb

    # feature permutation q = o*8+h  <->  f = h*64+o
    q = np.arange(D)
    f_of_q = (q % H) * O + (q // H)           # column f placed at position q
    Wfc_p = np.asarray(W_fc)[:, f_of_q]       # z_perm = src_h @ Wfc_p

    a_src = np.asarray(attn_a)[0, :, :O]       # [H, O]
    a_feat = np.asarray(attn_a)[0, :, 2 * O:]  # [H, O]
    Ablk = np.zeros((D, H), np.float32)
    for h in range(H):
        Ablk[h * O:(h + 1) * O, h] = a_src[h]
    M1 = (np.asarray(W_fc, np.float64) @ Ablk.astype(np.float64)).astype(np.float32)
    V = np.zeros((C.ed, H), np.float32)
    for h in range(H):
        V[:, h] = np.asarray(W_feat)[:, h * O:(h + 1) * O] @ a_feat[h]

    cores = []
    for c in range(C.ncores):
        idxw = np.zeros((C.nblk, 128, cpb * 8), np.int16)
        eeT = np.zeros((C.nblk, cpb, 128, 128), NP_BF16)
        oh = np.zeros((C.nblk, cpb, 128, 128), NP_BF16)
        th = np.zeros((C.tgt_pad, D), np.float32)
        for b in range(C.nblk):
            g = c * C.nblk + b
            s, t = bstart[g], bstart[g + 1]
            n = t - s
            if n > 0:
                src_b = np.zeros(cpb * 128, np.int64)
                src_b[:n] = es[s:t]
                lt = np.full(cpb * 128, -1, np.int64)
                lt[:n] = lt_all[s:t]
                # gather index wrap: logical i -> partition i%16, col i//16, x8
                base = src_b.astype(np.int16).reshape(-1, 16).T  # [16, cpb*8]
                for k in range(8):
                    idxw[b, k * 16:(k + 1) * 16, :] = base
                eb = np.zeros((cpb * 128, C.ed), NP_BF16)
                eb[:n] = ee[s:t].astype(NP_BF16)
                eeT[b] = eb.reshape(cpb, 128, C.ed).transpose(0, 2, 1)
                ohb = np.zeros((cpb * 128, 128), NP_BF16)
                valid = lt >= 0
                ohb[np.nonzero(valid)[0], lt[valid]] = 1.0
                oh[b] = ohb.reshape(cpb, 128, 128)
            # tgt_h rows for this block's slots (elu trick: + (tgt_h - 1))
            nodes = np.nonzero(block_of == g)[0]
            th[b * 128 + slot_of[nodes]] = np.asarray(tgt_h)[nodes] - 1.0

        sh = np.zeros((C.in_dim, C.tgt_pad), np.float32)
        lo2 = c * C.tgt_per
        hi2 = min((c + 1) * C.tgt_per, C.n_src)
        sh[:, :hi2 - lo2] = np.asarray(src_h)[lo2:hi2].T

        cores.append({
            "idxw": idxw, "eeT": eeT, "oh": oh,
            "tgt_hm1": th.astype(NP_BF16),
            "src_hT": sh.astype(NP_BF16),
        })

    # inverse permutation: node -> row in concatenated shard outputs
    row_of_node = (block_of // C.nblk) * C.tgt_pad + \
                  (block_of % C.nblk) * 128 + slot_of

    shared = {
        "wfc": Wfc_p.astype(NP_BF16),
        "m1": M1.astype(NP_BF16),
        "v": V.astype(NP_BF16),
        "w1": np.asarray(w1).astype(NP_BF16),
        "w2": np.asarray(w2).astype(NP_BF16),
        "b1c": np.asarray(b1, np.float32).reshape(C.fh, 1),
        "b2c": np.asarray(b2, np.float32).reshape(D, 1),
        "g_rep": np.tile(np.asarray(ln_g, np.float32).reshape(1, D), (128, 1)),
        "b_rep": np.tile(np.asarray(ln_b, np.float32).reshape(1, D), (128, 1)),
        "identb": np.eye(128, dtype=NP_BF16),
    }
    return cores, shared, row_of_node


def build_program(C):
    nc = bacc.Bacc("TRN2", target_bir_lowering=False, debug=False,
                   num_devices=C.ncores)
    H, O, D, CPB, NBLK = C.h, C.o, C.d, C.cpb, C.nblk
    ZR = C.zrow
    NPAD = C.tgt_pad
    BUFS = 4            # zg lookahead depth
    GCALL = 4           # gather chunks per swdge call (512 idxs)

    # -------- I/O --------
    def din(name, shape, dt):
        return nc.dram_tensor(name, shape, dt, kind="ExternalInput").ap()

    idxw = din("idxw", [NBLK, 128, CPB * 8], I16)
    eeT = din("eeT", [NBLK, CPB, 128, 128], BF16)
    oh = din("oh", [NBLK, CPB, 128, 128], BF16)
    tgt_hm1 = din("tgt_hm1", [NPAD, D], BF16)
    src_hT = din("src_hT", [C.in_dim, NPAD], BF16)
    wfc = din("wfc", [C.in_dim, D], BF16)
    m1 = din("m1", [C.in_dim, H], BF16)
    vmat = din("v", [C.ed, H], BF16)
    w1 = din("w1", [D, C.fh], BF16)
    w2 = din("w2", [C.fh, D], BF16)
    b1c = din("b1c", [C.fh, 1], F32)
    b2c = din("b2c", [D, 1], F32)
    g_rep = din("g_rep", [128, D], F32)
    b_rep = din("b_rep", [128, D], F32)
    identb = din("identb", [128, 128], BF16)

    out_shard = nc.dram_tensor("out_shard", [NPAD, D], F32,
                               kind="ExternalOutput").ap()

    zc_bounce = nc.dram_tensor("zc_bounce", [C.tgt_per, ZR], BF16).ap()
    zc_space = "Shared" if C.ncores > 4 else None
    zc_table = nc.dram_tensor("zc_table", [C.n_src, ZR], BF16,
                              addr_space=zc_space).ap()

    KT = C.in_dim // 128   # 4
    FT = D // 128          # 4
    MT1 = C.fh // 128      # 16
    chunks = [(s, min(512, NPAD - s)) for s in range(0, NPAD, 512)]

    with tile.TileContext(nc) as tc, ExitStack() as top:
        const = top.enter_context(tc.tile_pool(name="const", bufs=1))
        zgp = top.enter_context(tc.tile_pool(name="zg", bufs=BUFS))

        # idx table first: SWDGE preps consume it
        idx_sb = const.tile([128, NBLK, CPB * 8], I16)
        nc.sync.dma_start(idx_sb[:], idxw[:, :, :].rearrange("b p s -> p b s"))
        v_sb = const.tile([128, H], BF16)
        nc.sync.dma_start(v_sb[:], vmat[:, :])
        idb_sb = const.tile([128, 128], BF16)
        nc.sync.dma_start(idb_sb[:], identb[:, :])
        zero_sb = const.tile([128, 1], F32)
        nc.vector.memset(zero_sb[:], 0.0)
        eps_sb = const.tile([128, 1], F32)
        nc.vector.memset(eps_sb[:], LN_EPS)
        zpad_sb = const.tile([128, ZR - D - H], BF16)
        nc.vector.memset(zpad_sb[:], 0.0)
        # (w1/w2/ln consts DMA'd after phase 0 below; allocated now)
        w1_sb = const.tile([128, KT, MT1, 128], BF16)
        w2_sb = const.tile([128, MT1, FT, 128], BF16)
        b1_sb = const.tile([128, MT1, 1], F32)
        b2_sb = const.tile([128, FT, 1], F32)
        grep_sb = const.tile([128, D], F32)
        brep_sb = const.tile([128, D], F32)

        zg_tiles = {}

        def emit_gather(nb):
            """Gather issued BUFS blocks ahead of consumption so GpSimd
            desc-gen overlaps compute on the other engines."""
            zg = zgp.tile([128, CPB, ZR], BF16, tag="zg")
            zg_tiles[nb] = zg
            for g0 in range(0, CPB, GCALL):
                gn = min(GCALL, CPB - g0)
                nc.gpsimd.dma_gather(
                    out_ap=zg[:, g0:g0 + gn, :], in_ap=zc_table[:, :],
                    idxs_ap=idx_sb[:, nb, g0 * 8:(g0 + gn) * 8],
                    num_idxs=gn * 128, num_idxs_reg=gn * 128, elem_size=ZR)

        # ---------------- phase 0: z rows + s1 rows -> zc_bounce -> AllGather
        with ExitStack() as p0:
            ps0 = p0.enter_context(tc.tile_pool(name="ps0", bufs=2, space="PSUM"))
            zr_pool = p0.enter_context(tc.tile_pool(name="zrow", bufs=2))
            shp = p0.enter_context(tc.tile_pool(name="shp", bufs=1))
            sh_sb = shp.tile([128, KT, NPAD], BF16)
            nc.sync.dma_start(sh_sb[:],
                              src_hT.rearrange("(kt p) n -> p kt n", p=128))
            wfc_sb = shp.tile([128, KT, D], BF16)
            nc.sync.dma_start(wfc_sb[:], wfc.rearrange("(kt p) m -> p kt m", p=128))
            m1_sb = shp.tile([128, KT, H], BF16)
            nc.sync.dma_start(m1_sb[:], m1.rearrange("(kt p) m -> p kt m", p=128))
            for nb in range(NBLK):
                rows = min(128, C.tgt_per - nb * 128)
                if rows <= 0:
                    break
                z_ps = ps0.tile([128, D], F32, tag="zps")
                for kt in range(KT):
                    nc.tensor.matmul(z_ps[:], sh_sb[:, kt, nb * 128:(nb + 1) * 128],
                                     wfc_sb[:, kt, :], start=(kt == 0),
                                     stop=(kt == KT - 1))
                s1_ps = ps0.tile([128, H], F32, tag="s1ps")
                for kt in range(KT):
                    nc.tensor.matmul(s1_ps[:], sh_sb[:, kt, nb * 128:(nb + 1) * 128],
                                     m1_sb[:, kt, :], start=(kt == 0),
                                     stop=(kt == KT - 1))
                zrow = zr_pool.tile([128, D], BF16, tag="zrow")
                nc.vector.tensor_copy(zrow[:], z_ps[:])
                s1row = zr_pool.tile([128, H], BF16, tag="s1row")
                nc.vector.tensor_copy(s1row[:], s1_ps[:])
                nc.sync.dma_start(zc_bounce[nb * 128:nb * 128 + rows, 0:D],
                                  zrow[0:rows, :])
                nc.sync.dma_start(zc_bounce[nb * 128:nb * 128 + rows, D:D + H],
                                  s1row[0:rows, :])
                nc.sync.dma_start(zc_bounce[nb * 128:nb * 128 + rows, D + H:ZR],
                                  zpad_sb[0:rows, :])

        nc.gpsimd.collective_compute(
            "AllGather", mybir.AluOpType.bypass,
            replica_groups=[list(range(C.ncores))],
            ins=[zc_bounce[:, :]], outs=[zc_table[:, :]],
        )

        for nb0 in range(BUFS):
            emit_gather(nb0)

        # FFN / LN constants (not needed until the first FFN chunk)
        nc.sync.dma_start(w1_sb[:], w1.rearrange("(kt p) (mt m) -> p kt mt m",
                                                 p=128, m=128))
        nc.sync.dma_start(w2_sb[:], w2.rearrange("(kt p) (mt m) -> p kt mt m",
                                                 p=128, m=128))
        nc.sync.dma_start(b1_sb[:], b1c.rearrange("(mt p) x -> p mt x", p=128))
        nc.sync.dma_start(b2_sb[:], b2c.rearrange("(mt p) x -> p mt x", p=128))
        nc.sync.dma_start(grep_sb[:], g_rep[:, :])
        nc.sync.dma_start(brep_sb[:], b_rep[:, :])

        # ---------------- blocks + FFN, interleaved so PE fills gather gaps
        with ExitStack() as pb:
            ps_s2 = pb.enter_context(tc.tile_pool(name="ps_s2", bufs=1, space="PSUM"))
            ps_hag = pb.enter_context(tc.tile_pool(name="ps_hag", bufs=1, space="PSUM"))
            ps_tp = pb.enter_context(tc.tile_pool(name="ps_tp", bufs=1, space="PSUM"))
            ps_a1 = pb.enter_context(tc.tile_pool(name="ps_a1", bufs=2, space="PSUM"))
            ps_o2 = pb.enter_context(tc.tile_pool(name="ps_o2", bufs=2, space="PSUM"))
            gpool = pb.enter_context(tc.tile_pool(name="ee_oh", bufs=2))
            epool = pb.enter_context(tc.tile_pool(name="escore", bufs=2))
            hpool = pb.enter_context(tc.tile_pool(name="hb", bufs=1))
            tgtp = pb.enter_context(tc.tile_pool(name="tgtp", bufs=2))
            hbtp = pb.enter_context(tc.tile_pool(name="hbt", bufs=1))
            fpool = pb.enter_context(tc.tile_pool(name="ffn", bufs=1))
            r1p = pb.enter_context(tc.tile_pool(name="r1", bufs=1))
            tmpp = pb.enter_context(tc.tile_pool(name="tmp", bufs=2))
            lnp = pb.enter_context(tc.tile_pool(name="ln", bufs=1))
            stp = pb.enter_context(tc.tile_pool(name="stat", bufs=2))

            hbT = hbtp.tile([128, FT, NPAD], BF16)
            r2 = fpool.tile([128, FT, NPAD], BF16)

            def emit_ffn_chunk(cs, cw):
                r1 = r1p.tile([128, MT1, cw], BF16, tag="r1")
                for mt in range(MT1):
                    a1 = ps_a1.tile([128, cw], F32, tag="a1")
                    for kt in range(KT):
                        nc.tensor.matmul(a1[:], w1_sb[:, kt, mt, :],
                                         hbT[:, kt, cs:cs + cw],
                                         start=(kt == 0), stop=(kt == KT - 1))
                    if mt % 2 == 0:
                        nc.scalar.activation(r1[:, mt, :], a1[:],
                                             mybir.ActivationFunctionType.Relu,
                                             bias=b1_sb[:, mt, :])
                    else:
                        nc.vector.tensor_scalar(r1[:, mt, :], a1[:],
                                                b1_sb[:, mt, :], 0.0,
                                                mybir.AluOpType.add,
                                                mybir.AluOpType.max)
                for mt2 in range(FT):
                    o2 = ps_o2.tile([128, cw], F32, tag="o2")
                    for kt2 in range(MT1):
                        nc.tensor.matmul(o2[:], w2_sb[:, kt2, mt2, :],
                                         r1[:, kt2, :],
                                         start=(kt2 == 0), stop=(kt2 == MT1 - 1))
                    t1 = tmpp.tile([128, cw], F32, tag="t1")
                    nc.vector.tensor_scalar_add(t1[:], o2[:], b2_sb[:, mt2, :])
                    nc.vector.tensor_tensor(r2[:, mt2, cs:cs + cw], t1[:],
                                            hbT[:, mt2, cs:cs + cw],
                                            mybir.AluOpType.add)

            def emit_ln(nb):
                r2n = lnp.tile([128, D], BF16, tag="r2n")
                for ft in range(FT):
                    tp = ps_tp.tile([128, 128], BF16, tag="tp")
                    nc.tensor.transpose(tp[:], r2[:, ft, nb * 128:(nb + 1) * 128],
                                        idb_sb[:])
                    nc.vector.tensor_copy(r2n[:, ft * 128:(ft + 1) * 128], tp[:])
                scrap = lnp.tile([128, D], BF16, tag="scrap")
                ssum = stp.tile([128, 1], F32, tag="ssum")
                nc.scalar.activation(scrap[:], r2n[:],
                                     mybir.ActivationFunctionType.Copy,
                                     accum_out=ssum[:])
                mu = stp.tile([128, 1], F32, tag="mu")
                nc.vector.tensor_scalar_mul(mu[:], ssum[:], 1.0 / D)
                xc = lnp.tile([128, D], BF16, tag="xc")
                nc.vector.tensor_scalar(xc[:], r2n[:], mu[:], None,
                                        mybir.AluOpType.subtract)
                sq = lnp.tile([128, D], BF16, tag="sq")
                ssq = stp.tile([128, 1], F32, tag="ssq")
                nc.scalar.activation(sq[:], xc[:],
                                     mybir.ActivationFunctionType.Square,
                                     accum_out=ssq[:])
                std = stp.tile([128, 1], F32, tag="std")
                nc.scalar.activation(std[:], ssq[:],
                                     mybir.ActivationFunctionType.Sqrt,
                                     bias=eps_sb[:, :], scale=1.0 / D)
                rstd = stp.tile([128, 1], F32, tag="rstd")
                nc.vector.reciprocal(rstd[:], std[:])
                xn = lnp.tile([128, D], BF16, tag="xn")
                nc.vector.tensor_scalar_mul(xn[:], xc[:], rstd[:])
                xg = lnp.tile([128, D], F32, tag="xg")
                nc.vector.tensor_tensor(xg[:], xn[:], grep_sb[:],
                                        mybir.AluOpType.mult)
                orow = lnp.tile([128, D], F32, tag="orow")
                nc.vector.tensor_tensor(orow[:], xg[:], brep_sb[:],
                                        mybir.AluOpType.add)
                nc.sync.dma_start(out_shard[nb * 128:(nb + 1) * 128, :], orow[:])

            next_chunk = 0
            for nb in range(NBLK):
                zg = zg_tiles[nb]
                ee_t = gpool.tile([128, CPB, 128], BF16, tag="ee")
                nc.sync.dma_start(ee_t[:], eeT[nb].rearrange("c d e -> d c e"))
                oh_t = gpool.tile([128, CPB, 128], BF16, tag="oh")
                nc.sync.dma_start(oh_t[:], oh[nb].rearrange("c e t -> e c t"))
                tgtb = tgtp.tile([128, D], BF16, tag="tgtb")
                nc.sync.dma_start(tgtb[:], tgt_hm1[nb * 128:(nb + 1) * 128, :])

                s2_ps = ps_s2.tile([128, CPB * H], F32, tag="s2")
                for j in range(CPB):
                    nc.tensor.matmul(s2_ps[:, j * H:(j + 1) * H], ee_t[:, j, :],
                                     v_sb[:, :], start=True, stop=True)
                e1 = epool.tile([128, CPB, H], F32, tag="e1")
                nc.vector.tensor_tensor(
                    e1[:, :, :],
                    s2_ps[:, :].rearrange("p (c h) -> p c h", h=H),
                    zg[:, :, D:D + H],
                    mybir.AluOpType.add)
                lk = epool.tile([128, CPB, H], F32, tag="lk")
                nc.vector.tensor_scalar_mul(lk[:, :, :], e1[:, :, :], LEAK)
                e2 = epool.tile([128, CPB, H], F32, tag="e2")
                nc.vector.tensor_tensor(e2[:, :, :], e1[:, :, :], lk[:, :, :],
                                        mybir.AluOpType.max)
                eexp = epool.tile([128, CPB, H], BF16, tag="eexp")
                nc.scalar.activation(eexp[:, :, :], e2[:, :, :],
                                     mybir.ActivationFunctionType.Exp,
                                     bias=zero_sb[:, :])

                # eexp weighting in place into the gathered z columns
                nc.vector.tensor_tensor(
                    zg[:, :, 0:D].rearrange("p c (o h) -> p c o h", h=H),
                    zg[:, :, 0:D].rearrange("p c (o h) -> p c o h", h=H),
                    eexp[:, :, :].rearrange("p c (h x) -> p c x h", x=1)
                        .broadcast_to([128, CPB, O, H]),
                    mybir.AluOpType.mult)

                hag = ps_hag.tile([128, D + H], F32, tag="hag")
                for j in range(CPB):
                    nc.tensor.matmul(hag[:, 0:D], oh_t[:, j, :], zg[:, j, 0:D],
                                     start=(j == 0), stop=(j == CPB - 1),
                                     skip_group_check=True)
                    nc.tensor.matmul(hag[:, D:D + H], oh_t[:, j, :], eexp[:, j, :],
                                     start=(j == 0), stop=(j == CPB - 1),
                                     skip_group_check=True)

                den = epool.tile([128, H], F32, tag="den")
                nc.vector.tensor_scalar_max(den[:], hag[:, D:D + H], 1e-30)
                rec = epool.tile([128, H], F32, tag="rec")
                nc.vector.reciprocal(rec[:], den[:])

                hbp = hpool.tile([128, D], BF16, tag="hbp")
                nc.vector.tensor_tensor(
                    hbp[:, :].rearrange("p (h o) -> p h o", o=O),
                    hag[:, 0:D].rearrange("p (o h) -> p h o", h=H),
                    rec[:, :].rearrange("p (h x) -> p h x", x=1)
                        .broadcast_to([128, H, O]),
                    mybir.AluOpType.mult)
                # elu(x) + tgt = max(x,0) + min(exp(x),1) + (tgt-1)
                mx = hpool.tile([128, D], BF16, tag="mx")
                nc.scalar.activation(mx[:], hbp[:],
                                     mybir.ActivationFunctionType.Relu)
                ex = hpool.tile([128, D], BF16, tag="ex")
                nc.scalar.activation(ex[:], hbp[:],
                                     mybir.ActivationFunctionType.Exp,
                                     bias=zero_sb[:, :])
                ex1 = hpool.tile([128, D], BF16, tag="ex1")
                nc.vector.tensor_scalar_min(ex1[:], ex[:], 1.0)
                hb2 = hpool.tile([128, D], BF16, tag="hb2")
                nc.vector.tensor_tensor(hb2[:], ex1[:], mx[:], mybir.AluOpType.add)
                hb3 = hpool.tile([128, D], BF16, tag="hb3")
                nc.vector.tensor_tensor(hb3[:], hb2[:], tgtb[:],
                                        mybir.AluOpType.add)
                for ft in range(FT):
                    tpb = ps_tp.tile([128, 128], BF16, tag="tp")
                    nc.tensor.transpose(tpb[:], hb3[:, ft * 128:(ft + 1) * 128],
                                        idb_sb[:])
                    nc.vector.tensor_copy(hbT[:, ft, nb * 128:(nb + 1) * 128],
                                          tpb[:])

                # prep+fire gather nb+BUFS (its zg slot was freed by block nb)
                if nb + BUFS < NBLK:
                    emit_gather(nb + BUFS)

                while (next_chunk < len(chunks)
                       and chunks[next_chunk][0] + chunks[next_chunk][1]
                       <= (nb + 1) * 128):
                    cs, cw = chunks[next_chunk]
                    emit_ffn_chunk(cs, cw)
                    for nb_ln in range(cs // 128, (cs + cw) // 128):
                        emit_ln(nb_ln)
                    next_chunk += 1

    nc.compile()
    return nc


_CACHE = {}


def _get_program(C):
    key = (C.ncores, C.n_src, C.n_tgt, C.e, C.cpb)
    if key not in _CACHE:
        _CACHE[key] = build_program(C)
    return _CACHE[key]


def kernel(src_h, tgt_h, edge_embed, edge_src, edge_dst,
           W_fc, W_feat, attn_a, w1, b1, w2, b2, ln_g, ln_b):
    from concourse.bass_utils import run_bass_kernel_spmd

    C = full_cfg()
    cores, shared, row_of_node = host_prep(
        C, src_h, tgt_h, edge_embed, edge_src, edge_dst,
        W_fc, W_feat, attn_a, w1, b1, w2, b2, ln_g, ln_b)
    nc = _get_program(C)
    in_maps = []
    for c in range(C.ncores):
        m = dict(shared)
        cc = cores[c]
        m.update(idxw=cc["idxw"], eeT=cc["eeT"], oh=cc["oh"],
                 tgt_hm1=cc["tgt_hm1"], src_hT=cc["src_hT"])
        in_maps.append(m)
    import os
    try:
        res = run_bass_kernel_spmd(nc, in_maps, list(range(C.ncores)))
    except Exception:
        if os.environ.get("BASS_TRACE"):
            os.environ["BASS_NEVER_TRACE"] = "1"
            res = run_bass_kernel_spmd(nc, in_maps, list(range(C.ncores)))
        else:
            raise
    global _last_results
    _last_results = res
    allrows = np.concatenate(
        [res.results[c]["out_shard"] for c in range(C.ncores)], axis=0)
    out = allrows[row_of_node]
    return np.ascontiguousarray(out, dtype=np.float32)


# revision 28
# speedup vs baseline: 1.2709x; 1.0108x over previous
"""Trainium2 Bass kernel for a fused MultiHead-GAT layer (8-core SPMD).

Strategy (edges sharded by balanced dst blocks; tgt nodes data-parallel):
  host:  assign dst nodes to 80 edge-balanced blocks of <=128 nodes
         (10 blocks/core, CPB uniform), group edges by block, pre-transpose
         edge_embed chunks, pre-build one-hot (edge->local slot) chunks,
         fold attn_a into tiny weight matrices; final output rows are
         un-permuted on host.
  device (per core):
    z rows  = src_h_shard @ W_fc (columns permuted o*8+h), s1 = src_h @ (W_fc @ Ablk)
    AllGather -> full [N_SRC, 640] bf16 table (z|s1|pad)
    gathers run as SWDGE prepare_only descriptor-gen (hoisted into the
    preamble / overlapped with compute on GpSimd) + per-block trigger_dma;
    per block: s2 via PE (edge_embed^T @ V), e = leaky(s1+s2) on Scalar;
    softmax without max-shift; eexp weighting multiplied IN PLACE into the
    gathered z tile; aggregation via one-hot matmul into PSUM [t, 512+8];
    divide+unpermute, elu+residual (Scalar Relu/Exp + min(exp,1) trick);
    FFN (bf16 matmuls) feature-major, interleaved with the block loop;
    PE-transpose to node-major; LayerNorm per node (Scalar accum_out for
    the reductions); f32 output.
"""
import sys

sys.path.insert(0, "/opt/trn_rl_repo")

from contextlib import ExitStack
from types import SimpleNamespace

import numpy as np
import ml_dtypes

import concourse.bass as bass
import concourse.bacc as bacc
import concourse.tile as tile
from concourse import mybir

BF16 = mybir.dt.bfloat16
F32 = mybir.dt.float32
I16 = mybir.dt.int16
NP_BF16 = ml_dtypes.bfloat16

LN_EPS = 1e-5
LEAK = 0.01


def full_cfg():
    return SimpleNamespace(
        ncores=8,
        n_src=10000, n_tgt=10000, e=160000,
        in_dim=512, d=512, h=8, o=64, ed=128, fh=2048,
        tgt_per=1250, tgt_pad=1280, nblk=10,
        zrow=640,  # 512 z + 8 s1 + 120 pad (row bytes % 256 == 0)
    )


def _balance_blocks(deg, nblocks, slots_per_block):
    """Assign nodes to blocks, balancing total degree, <=slots nodes/block."""
    import heapq
    order = np.argsort(-deg, kind="stable")
    heap = [(0, b) for b in range(nblocks)]  # (edges, block)
    heapq.heapify(heap)
    nslots = np.zeros(nblocks, np.int64)
    block_of = np.empty(len(deg), np.int64)
    slot_of = np.empty(len(deg), np.int64)
    spill = []
    for n in order:
        while True:
            e_cnt, b = heapq.heappop(heap)
            if nslots[b] < slots_per_block:
                break
            spill.append((e_cnt, b))
        block_of[n] = b
        slot_of[n] = nslots[b]
        nslots[b] += 1
        heapq.heappush(heap, (e_cnt + int(deg[n]), b))
        while spill:
            heapq.heappush(heap, spill.pop())
    return block_of, slot_of


def host_prep(cfg, src_h, tgt_h, edge_embed, edge_src, edge_dst,
              W_fc, W_feat, attn_a, w1, b1, w2, b2, ln_g, ln_b):
    C = cfg
    H, O, D = C.h, C.o, C.d
    NBLOCKS = C.ncores * C.nblk

    deg = np.bincount(np.asarray(edge_dst), minlength=C.n_tgt)
    block_of, slot_of = _balance_blocks(deg, NBLOCKS, 128)

    eb_blk = block_of[np.asarray(edge_dst)]
    perm = np.argsort(eb_blk, kind="stable")
    es = np.asarray(edge_src)[perm].astype(np.int64)
    ed_blk = eb_blk[perm]
    lt_all = slot_of[np.asarray(edge_dst)][perm]
    ee = np.asarray(edge_embed)[perm]
    bstart = np.searchsorted(ed_blk, np.arange(NBLOCKS + 1))

    cnts = bstart[1:] - bstart[:-1]
    cpb = max(1, int((cnts.max() + 127) // 128))
    C.cpb = cpb

    # feature permutation q = o*8+h  <->  f = h*64+o
    q = np.arange(D)
    f_of_q = (q % H) * O + (q // H)           # column f placed at position q
    Wfc_p = np.asarray(W_fc)[:, f_of_q]       # z_perm = src_h @ Wfc_p

    a_src = np.asarray(attn_a)[0, :, :O]       # [H, O]
    a_feat = np.asarray(attn_a)[0, :, 2 * O:]  # [H, O]
    Ablk = np.zeros((D, H), np.float32)
    for h in range(H):
        Ablk[h * O:(h + 1) * O, h] = a_src[h]
    M1 = (np.asarray(W_fc, np.float64) @ Ablk.astype(np.float64)).astype(np.float32)
    V = np.zeros((C.ed, H), np.float32)
    for h in range(H):
        V[:, h] = np.asarray(W_feat)[:, h * O:(h + 1) * O] @ a_feat[h]

    cores = []
    for c in range(C.ncores):
        idxw = np.zeros((C.nblk, 128, cpb * 8), np.int16)
        eeT = np.zeros((C.nblk, cpb, 128, 128), NP_BF16)
        oh = np.zeros((C.nblk, cpb, 128, 128), NP_BF16)
        th = np.zeros((C.tgt_pad, D), np.float32)
        for b in range(C.nblk):
            g = c * C.nblk + b
            s, t = bstart[g], bstart[g + 1]
            n = t - s
            if n > 0:
                src_b = np.zeros(cpb * 128, np.int64)
                src_b[:n] = es[s:t]
                lt = np.full(cpb * 128, -1, np.int64)
                lt[:n] = lt_all[s:t]
                # gather index wrap: logical i -> partition i%16, col i//16, x8
                base = src_b.astype(np.int16).reshape(-1, 16).T  # [16, cpb*8]
                for k in range(8):
                    idxw[b, k * 16:(k + 1) * 16, :] = base
                eb = np.zeros((cpb * 128, C.ed), NP_BF16)
                eb[:n] = ee[s:t].astype(NP_BF16)
                eeT[b] = eb.reshape(cpb, 128, C.ed).transpose(0, 2, 1)
                ohb = np.zeros((cpb * 128, 128), NP_BF16)
                valid = lt >= 0
                ohb[np.nonzero(valid)[0], lt[valid]] = 1.0
                oh[b] = ohb.reshape(cpb, 128, 128)
            # tgt_h rows for this block's slots (elu trick: + (tgt_h - 1))
            nodes = np.nonzero(block_of == g)[0]
            th[b * 128 + slot_of[nodes]] = np.asarray(tgt_h)[nodes] - 1.0

        sh = np.zeros((C.in_dim, C.tgt_pad), np.float32)
        lo2 = c * C.tgt_per
        hi2 = min((c + 1) * C.tgt_per, C.n_src)
        sh[:, :hi2 - lo2] = np.asarray(src_h)[lo2:hi2].T

        cores.append({
            "idxw": idxw, "eeT": eeT, "oh": oh,
            "tgt_hm1": th.astype(NP_BF16),
            "src_hT": sh.astype(NP_BF16),
        })

    # inverse permutation: node -> row in concatenated shard outputs
    row_of_node = (block_of // C.nblk) * C.tgt_pad + \
                  (block_of % C.nblk) * 128 + slot_of

    shared = {
        "wfc": Wfc_p.astype(NP_BF16),
        "m1": M1.astype(NP_BF16),
        "v": V.astype(NP_BF16),
        "w1": np.asarray(w1).astype(NP_BF16),
        "w2": np.asarray(w2).astype(NP_BF16),
        "b1c": np.asarray(b1, np.float32).reshape(C.fh, 1),
        "b2c": np.asarray(b2, np.float32).reshape(D, 1),
        "g_rep": np.tile(np.asarray(ln_g, np.float32).reshape(1, D), (128, 1)),
        "b_rep": np.tile(np.asarray(ln_b, np.float32).reshape(1, D), (128, 1)),
        "identb": np.eye(128, dtype=NP_BF16),
    }
    return cores, shared, row_of_node


def build_program(C):
    nc = bacc.Bacc("TRN2", target_bir_lowering=False, debug=False,
                   num_devices=C.ncores)
    H, O, D, CPB, NBLK = C.h, C.o, C.d, C.cpb, C.nblk
    ZR = C.zrow
    NPAD = C.tgt_pad
    BUFS = 4            # zg lookahead depth
    GCALL = 4           # gather chunks per swdge call (512 idxs)

    # -------- I/O --------
    def din(name, shape, dt):
        return nc.dram_tensor(name, shape, dt, kind="ExternalInput").ap()

    idxw = din("idxw", [NBLK, 128, CPB * 8], I16)
    eeT = din("eeT", [NBLK, CPB, 128, 128], BF16)
    oh = din("oh", [NBLK, CPB, 128, 128], BF16)
    tgt_hm1 = din("tgt_hm1", [NPAD, D], BF16)
    src_hT = din("src_hT", [C.in_dim, NPAD], BF16)
    wfc = din("wfc", [C.in_dim, D], BF16)
    m1 = din("m1", [C.in_dim, H], BF16)
    vmat = din("v", [C.ed, H], BF16)
    w1 = din("w1", [D, C.fh], BF16)
    w2 = din("w2", [C.fh, D], BF16)
    b1c = din("b1c", [C.fh, 1], F32)
    b2c = din("b2c", [D, 1], F32)
    g_rep = din("g_rep", [128, D], F32)
    b_rep = din("b_rep", [128, D], F32)
    identb = din("identb", [128, 128], BF16)

    out_shard = nc.dram_tensor("out_shard", [NPAD, D], F32,
                               kind="ExternalOutput").ap()

    zc_bounce = nc.dram_tensor("zc_bounce", [C.tgt_per, ZR], BF16).ap()
    zc_space = "Shared" if C.ncores > 4 else None
    zc_table = nc.dram_tensor("zc_table", [C.n_src, ZR], BF16,
                              addr_space=zc_space).ap()

    KT = C.in_dim // 128   # 4
    FT = D // 128          # 4
    MT1 = C.fh // 128      # 16
    chunks = [(s, min(512, NPAD - s)) for s in range(0, NPAD, 512)]

    with tile.TileContext(nc) as tc, ExitStack() as top:
        const = top.enter_context(tc.tile_pool(name="const", bufs=1))
        zgp = top.enter_context(tc.tile_pool(name="zg", bufs=BUFS))

        # idx table first: SWDGE preps consume it
        idx_sb = const.tile([128, NBLK, CPB * 8], I16)
        nc.sync.dma_start(idx_sb[:], idxw[:, :, :].rearrange("b p s -> p b s"))
        v_sb = const.tile([128, H], BF16)
        nc.sync.dma_start(v_sb[:], vmat[:, :])
        idb_sb = const.tile([128, 128], BF16)
        nc.sync.dma_start(idb_sb[:], identb[:, :])
        zero_sb = const.tile([128, 1], F32)
        nc.vector.memset(zero_sb[:], 0.0)
        eps_sb = const.tile([128, 1], F32)
        nc.vector.memset(eps_sb[:], LN_EPS)
        zpad_sb = const.tile([128, ZR - D - H], BF16)
        nc.vector.memset(zpad_sb[:], 0.0)
        # (w1/w2/ln consts DMA'd after phase 0 below; allocated now)
        w1_sb = const.tile([128, KT, MT1, 128], BF16)
        w2_sb = const.tile([128, MT1, FT, 128], BF16)
        b1_sb = const.tile([128, MT1, 1], F32)
        b2_sb = const.tile([128, FT, 1], F32)
        grep_sb = const.tile([128, D], F32)
        brep_sb = const.tile([128, D], F32)

        zg_tiles = {}

        def emit_gather(nb):
            """Gather issued BUFS blocks ahead of consumption so GpSimd
            desc-gen overlaps compute on the other engines."""
            zg = zgp.tile([128, CPB, ZR], BF16, tag="zg")
            zg_tiles[nb] = zg
            for g0 in range(0, CPB, GCALL):
                gn = min(GCALL, CPB - g0)
                nc.gpsimd.dma_gather(
                    out_ap=zg[:, g0:g0 + gn, :], in_ap=zc_table[:, :],
                    idxs_ap=idx_sb[:, nb, g0 * 8:(g0 + gn) * 8],
                    num_idxs=gn * 128, num_idxs_reg=gn * 128, elem_size=ZR)

        # ---------------- phase 0: z rows + s1 rows -> zc_bounce -> AllGather
        with ExitStack() as p0:
            ps0 = p0.enter_context(tc.tile_pool(name="ps0", bufs=2, space="PSUM"))
            zr_pool = p0.enter_context(tc.tile_pool(name="zrow", bufs=2))
            shp = p0.enter_context(tc.tile_pool(name="shp", bufs=1))
            sh_sb = shp.tile([128, KT, NPAD], BF16)
            nc.sync.dma_start(sh_sb[:],
                              src_hT.rearrange("(kt p) n -> p kt n", p=128))
            wfc_sb = shp.tile([128, KT, D], BF16)
            nc.sync.dma_start(wfc_sb[:], wfc.rearrange("(kt p) m -> p kt m", p=128))
            m1_sb = shp.tile([128, KT, H], BF16)
            nc.sync.dma_start(m1_sb[:], m1.rearrange("(kt p) m -> p kt m", p=128))
            for nb in range(NBLK):
                rows = min(128, C.tgt_per - nb * 128)
                if rows <= 0:
                    break
                z_ps = ps0.tile([128, D], F32, tag="zps")
                for kt in range(KT):
                    nc.tensor.matmul(z_ps[:], sh_sb[:, kt, nb * 128:(nb + 1) * 128],
                                     wfc_sb[:, kt, :], start=(kt == 0),
                                     stop=(kt == KT - 1))
                s1_ps = ps0.tile([128, H], F32, tag="s1ps")
                for kt in range(KT):
                    nc.tensor.matmul(s1_ps[:], sh_sb[:, kt, nb * 128:(nb + 1) * 128],
                                     m1_sb[:, kt, :], start=(kt == 0),
                                     stop=(kt == KT - 1))
                zrow = zr_pool.tile([128, D], BF16, tag="zrow")
                nc.vector.tensor_copy(zrow[:], z_ps[:])
                s1row = zr_pool.tile([128, H], BF16, tag="s1row")
                nc.vector.tensor_copy(s1row[:], s1_ps[:])
                nc.sync.dma_start(zc_bounce[nb * 128:nb * 128 + rows, 0:D],
                                  zrow[0:rows, :])
                nc.sync.dma_start(zc_bounce[nb * 128:nb * 128 + rows, D:D + H],
                                  s1row[0:rows, :])
                nc.sync.dma_start(zc_bounce[nb * 128:nb * 128 + rows, D + H:ZR],
                                  zpad_sb[0:rows, :])

        nc.gpsimd.collective_compute(
            "AllGather", mybir.AluOpType.bypass,
            replica_groups=[list(range(C.ncores))],
            ins=[zc_bounce[:, :]], outs=[zc_table[:, :]],
        )

        for nb0 in range(BUFS):
            emit_gather(nb0)

        # FFN / LN constants (not needed until the first FFN chunk)
        nc.sync.dma_start(w1_sb[:], w1.rearrange("(kt p) (mt m) -> p kt mt m",
                                                 p=128, m=128))
        nc.sync.dma_start(w2_sb[:], w2.rearrange("(kt p) (mt m) -> p kt mt m",
                                                 p=128, m=128))
        nc.sync.dma_start(b1_sb[:], b1c.rearrange("(mt p) x -> p mt x", p=128))
        nc.sync.dma_start(b2_sb[:], b2c.rearrange("(mt p) x -> p mt x", p=128))
        nc.sync.dma_start(grep_sb[:], g_rep[:, :])
        nc.sync.dma_start(brep_sb[:], b_rep[:, :])

        # ---------------- blocks + FFN, interleaved so PE fills gather gaps
        with ExitStack() as pb:
            ps_s2 = pb.enter_context(tc.tile_pool(name="ps_s2", bufs=1, space="PSUM"))
            ps_hag = pb.enter_context(tc.tile_pool(name="ps_hag", bufs=1, space="PSUM"))
            ps_tp = pb.enter_context(tc.tile_pool(name="ps_tp", bufs=1, space="PSUM"))
            ps_a1 = pb.enter_context(tc.tile_pool(name="ps_a1", bufs=2, space="PSUM"))
            ps_o2 = pb.enter_context(tc.tile_pool(name="ps_o2", bufs=2, space="PSUM"))
            gpool = pb.enter_context(tc.tile_pool(name="ee_oh", bufs=2))
            epool = pb.enter_context(tc.tile_pool(name="escore", bufs=2))
            hpool = pb.enter_context(tc.tile_pool(name="hb", bufs=1))
            tgtp = pb.enter_context(tc.tile_pool(name="tgtp", bufs=2))
            hbtp = pb.enter_context(tc.tile_pool(name="hbt", bufs=1))
            fpool = pb.enter_context(tc.tile_pool(name="ffn", bufs=1))
            r1p = pb.enter_context(tc.tile_pool(name="r1", bufs=1))
            tmpp = pb.enter_context(tc.tile_pool(name="tmp", bufs=2))
            lnp = pb.enter_context(tc.tile_pool(name="ln", bufs=1))
            stp = pb.enter_context(tc.tile_pool(name="stat", bufs=2))

            hbT = hbtp.tile([128, FT, NPAD], BF16)
            r2 = fpool.tile([128, FT, NPAD], BF16)

            def emit_ffn_chunk(cs, cw):
                r1 = r1p.tile([128, MT1, cw], BF16, tag="r1")
                for mt in range(MT1):
                    a1 = ps_a1.tile([128, cw], F32, tag="a1")
                    for kt in range(KT):
                        nc.tensor.matmul(a1[:], w1_sb[:, kt, mt, :],
                                         hbT[:, kt, cs:cs + cw],
                                         start=(kt == 0), stop=(kt == KT - 1))
                    if mt % 2 == 0:
                        nc.scalar.activation(r1[:, mt, :], a1[:],
                                             mybir.ActivationFunctionType.Relu,
                                             bias=b1_sb[:, mt, :])
                    else:
                        nc.vector.tensor_scalar(r1[:, mt, :], a1[:],
                                                b1_sb[:, mt, :], 0.0,
                                                mybir.AluOpType.add,
                                                mybir.AluOpType.max)
                for mt2 in range(FT):
                    o2 = ps_o2.tile([128, cw], F32, tag="o2")
                    for kt2 in range(MT1):
                        nc.tensor.matmul(o2[:], w2_sb[:, kt2, mt2, :],
                                         r1[:, kt2, :],
                                         start=(kt2 == 0), stop=(kt2 == MT1 - 1))
                    t1 = tmpp.tile([128, cw], F32, tag="t1")
                    nc.vector.tensor_scalar_add(t1[:], o2[:], b2_sb[:, mt2, :])
                    nc.vector.tensor_tensor(r2[:, mt2, cs:cs + cw], t1[:],
                                            hbT[:, mt2, cs:cs + cw],
                                            mybir.AluOpType.add)

            def emit_ln(nb):
                r2n = lnp.tile([128, D], BF16, tag="r2n")
                for ft in range(FT):
                    tp = ps_tp.tile([128, 128], BF16, tag="tp")
                    nc.tensor.transpose(tp[:], r2[:, ft, nb * 128:(nb + 1) * 128],
                                        idb_sb[:])
                    nc.vector.tensor_copy(r2n[:, ft * 128:(ft + 1) * 128], tp[:])
                scrap = lnp.tile([128, D], BF16, tag="scrap")
                ssum = stp.tile([128, 1], F32, tag="ssum")
                nc.scalar.activation(scrap[:], r2n[:],
                                     mybir.ActivationFunctionType.Copy,
                                     accum_out=ssum[:])
                mu = stp.tile([128, 1], F32, tag="mu")
                nc.vector.tensor_scalar_mul(mu[:], ssum[:], 1.0 / D)
                xc = lnp.tile([128, D], BF16, tag="xc")
                nc.vector.tensor_scalar(xc[:], r2n[:], mu[:], None,
                                        mybir.AluOpType.subtract)
                sq = lnp.tile([128, D], BF16, tag="sq")
                ssq = stp.tile([128, 1], F32, tag="ssq")
                nc.scalar.activation(sq[:], xc[:],
                                     mybir.ActivationFunctionType.Square,
                                     accum_out=ssq[:])
                std = stp.tile([128, 1], F32, tag="std")
                nc.scalar.activation(std[:], ssq[:],
                                     mybir.ActivationFunctionType.Sqrt,
                                     bias=eps_sb[:, :], scale=1.0 / D)
                rstd = stp.tile([128, 1], F32, tag="rstd")
                nc.vector.reciprocal(rstd[:], std[:])
                xn = lnp.tile([128, D], BF16, tag="xn")
                nc.vector.tensor_scalar_mul(xn[:], xc[:], rstd[:])
                xg = lnp.tile([128, D], F32, tag="xg")
                nc.vector.tensor_tensor(xg[:], xn[:], grep_sb[:],
                                        mybir.AluOpType.mult)
                orow = lnp.tile([128, D], F32, tag="orow")
                nc.vector.tensor_tensor(orow[:], xg[:], brep_sb[:],
                                        mybir.AluOpType.add)
                nc.sync.dma_start(out_shard[nb * 128:(nb + 1) * 128, :], orow[:])

            next_chunk = 0
            for nb in range(NBLK):
                zg = zg_tiles[nb]
                ee_t = gpool.tile([128, CPB, 128], BF16, tag="ee")
                nc.sync.dma_start(ee_t[:], eeT[nb].rearrange("c d e -> d c e"))
                oh_t = gpool.tile([128, CPB, 128], BF16, tag="oh")
                nc.sync.dma_start(oh_t[:], oh[nb].rearrange("c e t -> e c t"))
                tgtb = tgtp.tile([128, D], BF16, tag="tgtb")
                nc.sync.dma_start(tgtb[:], tgt_hm1[nb * 128:(nb + 1) * 128, :])

                s2_ps = ps_s2.tile([128, CPB * H], F32, tag="s2")
                for j in range(CPB):
                    nc.tensor.matmul(s2_ps[:, j * H:(j + 1) * H], ee_t[:, j, :],
                                     v_sb[:, :], start=True, stop=True)
                e1 = epool.tile([128, CPB, H], F32, tag="e1")
                nc.vector.tensor_tensor(
                    e1[:, :, :],
                    s2_ps[:, :].rearrange("p (c h) -> p c h", h=H),
                    zg[:, :, D:D + H],
                    mybir.AluOpType.add)
                lk = epool.tile([128, CPB, H], F32, tag="lk")
                nc.vector.tensor_scalar_mul(lk[:, :, :], e1[:, :, :], LEAK)
                e2 = epool.tile([128, CPB, H], F32, tag="e2")
                nc.vector.tensor_tensor(e2[:, :, :], e1[:, :, :], lk[:, :, :],
                                        mybir.AluOpType.max)
                eexp = epool.tile([128, CPB, H], BF16, tag="eexp")
                nc.scalar.activation(eexp[:, :, :], e2[:, :, :],
                                     mybir.ActivationFunctionType.Exp,
                                     bias=zero_sb[:, :])

                # eexp weighting in place into the gathered z columns
                nc.vector.tensor_tensor(
                    zg[:, :, 0:D].rearrange("p c (o h) -> p c o h", h=H),
                    zg[:, :, 0:D].rearrange("p c (o h) -> p c o h", h=H),
                    eexp[:, :, :].rearrange("p c (h x) -> p c x h", x=1)
                        .broadcast_to([128, CPB, O, H]),
                    mybir.AluOpType.mult)

                hag = ps_hag.tile([128, D + H], F32, tag="hag")
                for j in range(CPB):
                    nc.tensor.matmul(hag[:, 0:D], oh_t[:, j, :], zg[:, j, 0:D],
                                     start=(j == 0), stop=(j == CPB - 1),
                                     skip_group_check=True)
                    nc.tensor.matmul(hag[:, D:D + H], oh_t[:, j, :], eexp[:, j, :],
                                     start=(j == 0), stop=(j == CPB - 1),
                                     skip_group_check=True)

                den = epool.tile([128, H], F32, tag="den")
                nc.vector.tensor_scalar_max(den[:], hag[:, D:D + H], 1e-30)
                rec = epool.tile([128, H], F32, tag="rec")
                nc.vector.reciprocal(rec[:], den[:])

                hbp = hpool.tile([128, D], BF16, tag="hbp")
                nc.vector.tensor_tensor(
                    hbp[:, :].rearrange("p (h o) -> p h o", o=O),
                    hag[:, 0:D].rearrange("p (o h) -> p h o", h=H),
                    rec[:, :].rearrange("p (h x) -> p h x", x=1)
                        .broadcast_to([128, H, O]),
                    mybir.AluOpType.mult)
                # elu(x) + tgt = max(x,0) + min(exp(x),1) + (tgt-1)
                mx = hpool.tile([128, D], BF16, tag="mx")
                nc.scalar.activation(mx[:], hbp[:],
                                     mybir.ActivationFunctionType.Relu)
                ex = hpool.tile([128, D], BF16, tag="ex")
                nc.scalar.activation(ex[:], hbp[:],
                                     mybir.ActivationFunctionType.Exp,
                                     bias=zero_sb[:, :])
                ex1 = hpool.tile([128, D], BF16, tag="ex1")
                nc.vector.tensor_scalar_min(ex1[:], ex[:], 1.0)
                hb2 = hpool.tile([128, D], BF16, tag="hb2")
                nc.vector.tensor_tensor(hb2[:], ex1[:], mx[:], mybir.AluOpType.add)
                hb3 = hpool.tile([128, D], BF16, tag="hb3")
                nc.vector.tensor_tensor(hb3[:], hb2[:], tgtb[:],
                                        mybir.AluOpType.add)
                for ft in range(FT):
                    tpb = ps_tp.tile([128, 128], BF16, tag="tp")
                    nc.tensor.transpose(tpb[:], hb3[:, ft * 128:(ft + 1) * 128],
                                        idb_sb[:])
                    nc.vector.tensor_copy(hbT[:, ft, nb * 128:(nb + 1) * 128],
                                          tpb[:])

                # prep+fire gather nb+BUFS (its zg slot was freed by block nb)
                if nb + BUFS < NBLK:
                    emit_gather(nb + BUFS)

                while (next_chunk < len(chunks)
                       and chunks[next_chunk][0] + chunks[next_chunk][1]
                       <= (nb + 1) * 128):
                    cs, cw = chunks[next_chunk]
                    emit_ffn_chunk(cs, cw)
                    for nb_ln in range(cs // 128, (cs + cw) // 128):
                        emit_ln(nb_ln)
                    next_chunk += 1

    nc.compile()
    return nc


_CACHE = {}


def _get_program(C):
    key = (C.ncores, C.n_src, C.n_tgt, C.e, C.cpb)
    if key not in _CACHE:
        _CACHE[key] = build_program(C)
    return _CACHE[key]


def kernel(src_h, tgt_h, edge_embed, edge_src, edge_dst,
           W_fc, W_feat, attn_a, w1, b1, w2, b2, ln_g, ln_b):
    from concourse.bass_utils import run_bass_kernel_spmd

    C = full_cfg()
    cores, shared, row_of_node = host_prep(
        C, src_h, tgt_h, edge_embed, edge_src, edge_dst,
        W_fc, W_feat, attn_a, w1, b1, w2, b2, ln_g, ln_b)
    nc = _get_program(C)
    in_maps = []
    for c in range(C.ncores):
        m = dict(shared)
        cc = cores[c]
        m.update(idxw=cc["idxw"], eeT=cc["eeT"], oh=cc["oh"],
                 tgt_hm1=cc["tgt_hm1"], src_hT=cc["src_hT"])
        in_maps.append(m)
    import os
    try:
        res = run_bass_kernel_spmd(nc, in_maps, list(range(C.ncores)))
    except Exception:
        if os.environ.get("BASS_TRACE"):
            os.environ["BASS_NEVER_TRACE"] = "1"
            res = run_bass_kernel_spmd(nc, in_maps, list(range(C.ncores)))
        else:
            raise
    global _last_results
    _last_results = res
    allrows = np.concatenate(
        [res.results[c]["out_shard"] for c in range(C.ncores)], axis=0)
    out = allrows[row_of_node]
    return np.ascontiguousarray(out, dtype=np.float32)


# revision 30
# speedup vs baseline: 1.2748x; 1.0031x over previous
"""Trainium2 Bass kernel for a fused MultiHead-GAT layer (8-core SPMD).

Strategy (edges sharded by balanced dst blocks; tgt nodes data-parallel):
  host:  assign dst nodes to 80 edge-balanced blocks of <=128 nodes
         (10 blocks/core, CPB uniform), group edges by block, pre-transpose
         edge_embed chunks, pre-build one-hot (edge->local slot) chunks,
         fold attn_a into tiny weight matrices; final output rows are
         un-permuted on host.
  device (per core):
    z rows  = src_h_shard @ W_fc (columns permuted o*8+h), s1 = src_h @ (W_fc @ Ablk)
    AllGather -> full [N_SRC, 640] bf16 table (z|s1|pad)
    gathers run as SWDGE prepare_only descriptor-gen (hoisted into the
    preamble / overlapped with compute on GpSimd) + per-block trigger_dma;
    per block: s2 via PE (edge_embed^T @ V), e = leaky(s1+s2) on Scalar;
    softmax without max-shift; eexp weighting multiplied IN PLACE into the
    gathered z tile; aggregation via one-hot matmul into PSUM [t, 512+8];
    divide+unpermute, elu+residual (Scalar Relu/Exp + min(exp,1) trick);
    FFN (bf16 matmuls) feature-major, interleaved with the block loop;
    PE-transpose to node-major; LayerNorm per node (Scalar accum_out for
    the reductions); f32 output.

Measured on 8 axon trn2 cores: HW exec ~450 us (baseline 490-540 us),
max rel err 0.0086 vs a float64 oracle. The block loop is paced by
SWDGE gather descriptor-gen on GpSimd (~9 ns/row, ~18 us/block); the
4-deep zg lookahead overlaps it with Tensor/Vector compute. A
prepare_only/trigger_dma variant (desc-gen during the AllGather) was
tried and reverted: Tile does not gate consumers of prepare_only
gathers, and its swdge-lane sem accounting conflicts with user sems.
"""
import sys

sys.path.insert(0, "/opt/trn_rl_repo")

from contextlib import ExitStack
from types import SimpleNamespace

import numpy as np
import ml_dtypes

import concourse.bass as bass
import concourse.bacc as bacc
import concourse.tile as tile
from concourse import mybir

BF16 = mybir.dt.bfloat16
F32 = mybir.dt.float32
I16 = mybir.dt.int16
NP_BF16 = ml_dtypes.bfloat16

LN_EPS = 1e-5
LEAK = 0.01


def full_cfg():
    return SimpleNamespace(
        ncores=8,
        n_src=10000, n_tgt=10000, e=160000,
        in_dim=512, d=512, h=8, o=64, ed=128, fh=2048,
        tgt_per=1250, tgt_pad=1280, nblk=10,
        zrow=640,  # 512 z + 8 s1 + 120 pad (row bytes % 256 == 0)
    )


def _balance_blocks(deg, nblocks, slots_per_block):
    """Assign nodes to blocks, balancing total degree, <=slots nodes/block."""
    import heapq
    order = np.argsort(-deg, kind="stable")
    heap = [(0, b) for b in range(nblocks)]  # (edges, block)
    heapq.heapify(heap)
    nslots = np.zeros(nblocks, np.int64)
    block_of = np.empty(len(deg), np.int64)
    slot_of = np.empty(len(deg), np.int64)
    spill = []
    for n in order:
        while True:
            e_cnt, b = heapq.heappop(heap)
            if nslots[b] < slots_per_block:
                break
            spill.append((e_cnt, b))
        block_of[n] = b
        slot_of[n] = nslots[b]
        nslots[b] += 1
        heapq.heappush(heap, (e_cnt + int(deg[n]), b))
        while spill:
            heapq.heappush(heap, spill.pop())
    return block_of, slot_of


def host_prep(cfg, src_h, tgt_h, edge_embed, edge_src, edge_dst,
              W_fc, W_feat, attn_a, w1, b1, w2, b2, ln_g, ln_b):
    C = cfg
    H, O, D = C.h, C.o, C.d
    NBLOCKS = C.ncores * C.nblk

    deg = np.bincount(np.asarray(edge_dst), minlength=C.n_tgt)
    block_of, slot_of = _balance_blocks(deg, NBLOCKS, 128)

    eb_blk = block_of[np.asarray(edge_dst)]
    perm = np.argsort(eb_blk, kind="stable")
    es = np.asarray(edge_src)[perm].astype(np.int64)
    ed_blk = eb_blk[perm]
    lt_all = slot_of[np.asarray(edge_dst)][perm]
    ee = np.asarray(edge_embed)[perm]
    bstart = np.searchsorted(ed_blk, np.arange(NBLOCKS + 1))

    cnts = bstart[1:] - bstart[:-1]
    cpb = max(1, int((cnts.max() + 127) // 128))
    C.cpb = cpb

    # feature permutation q = o*8+h  <->  f = h*64+o
    q = np.arange(D)
    f_of_q = (q % H) * O + (q // H)           # column f placed at position q
    Wfc_p = np.asarray(W_fc)[:, f_of_q]       # z_perm = src_h @ Wfc_p

    a_src = np.asarray(attn_a)[0, :, :O]       # [H, O]
    a_feat = np.asarray(attn_a)[0, :, 2 * O:]  # [H, O]
    Ablk = np.zeros((D, H), np.float32)
    for h in range(H):
        Ablk[h * O:(h + 1) * O, h] = a_src[h]
    M1 = (np.asarray(W_fc, np.float64) @ Ablk.astype(np.float64)).astype(np.float32)
    V = np.zeros((C.ed, H), np.float32)
    for h in range(H):
        V[:, h] = np.asarray(W_feat)[:, h * O:(h + 1) * O] @ a_feat[h]

    cores = []
    for c in range(C.ncores):
        idxw = np.zeros((C.nblk, 128, cpb * 8), np.int16)
        eeT = np.zeros((C.nblk, cpb, 128, 128), NP_BF16)
        oh = np.zeros((C.nblk, cpb, 128, 128), NP_BF16)
        th = np.zeros((C.tgt_pad, D), np.float32)
        for b in range(C.nblk):
            g = c * C.nblk + b
            s, t = bstart[g], bstart[g + 1]
            n = t - s
            if n > 0:
                src_b = np.zeros(cpb * 128, np.int64)
                src_b[:n] = es[s:t]
                lt = np.full(cpb * 128, -1, np.int64)
                lt[:n] = lt_all[s:t]
                # gather index wrap: logical i -> partition i%16, col i//16, x8
                base = src_b.astype(np.int16).reshape(-1, 16).T  # [16, cpb*8]
                for k in range(8):
                    idxw[b, k * 16:(k + 1) * 16, :] = base
                eb = np.zeros((cpb * 128, C.ed), NP_BF16)
                eb[:n] = ee[s:t].astype(NP_BF16)
                eeT[b] = eb.reshape(cpb, 128, C.ed).transpose(0, 2, 1)
                ohb = np.zeros((cpb * 128, 128), NP_BF16)
                valid = lt >= 0
                ohb[np.nonzero(valid)[0], lt[valid]] = 1.0
                oh[b] = ohb.reshape(cpb, 128, 128)
            # tgt_h rows for this block's slots (elu trick: + (tgt_h - 1))
            nodes = np.nonzero(block_of == g)[0]
            th[b * 128 + slot_of[nodes]] = np.asarray(tgt_h)[nodes] - 1.0

        sh = np.zeros((C.in_dim, C.tgt_pad), np.float32)
        lo2 = c * C.tgt_per
        hi2 = min((c + 1) * C.tgt_per, C.n_src)
        sh[:, :hi2 - lo2] = np.asarray(src_h)[lo2:hi2].T

        cores.append({
            "idxw": idxw, "eeT": eeT, "oh": oh,
            "tgt_hm1": th.astype(NP_BF16),
            "src_hT": sh.astype(NP_BF16),
        })

    # inverse permutation: node -> row in concatenated shard outputs
    row_of_node = (block_of // C.nblk) * C.tgt_pad + \
                  (block_of % C.nblk) * 128 + slot_of

    shared = {
        "wfc": Wfc_p.astype(NP_BF16),
        "m1": M1.astype(NP_BF16),
        "v": V.astype(NP_BF16),
        "w1": np.asarray(w1).astype(NP_BF16),
        "w2": np.asarray(w2).astype(NP_BF16),
        "b1c": np.asarray(b1, np.float32).reshape(C.fh, 1),
        "b2c": np.asarray(b2, np.float32).reshape(D, 1),
        "g_rep": np.tile(np.asarray(ln_g, np.float32).reshape(1, D), (128, 1)),
        "b_rep": np.tile(np.asarray(ln_b, np.float32).reshape(1, D), (128, 1)),
        "identb": np.eye(128, dtype=NP_BF16),
    }
    return cores, shared, row_of_node


def build_program(C):
    nc = bacc.Bacc("TRN2", target_bir_lowering=False, debug=False,
                   num_devices=C.ncores)
    H, O, D, CPB, NBLK = C.h, C.o, C.d, C.cpb, C.nblk
    ZR = C.zrow
    NPAD = C.tgt_pad
    BUFS = 4            # zg lookahead depth
    GCALL = 4           # gather chunks per swdge call (512 idxs)

    # -------- I/O --------
    def din(name, shape, dt):
        return nc.dram_tensor(name, shape, dt, kind="ExternalInput").ap()

    idxw = din("idxw", [NBLK, 128, CPB * 8], I16)
    eeT = din("eeT", [NBLK, CPB, 128, 128], BF16)
    oh = din("oh", [NBLK, CPB, 128, 128], BF16)
    tgt_hm1 = din("tgt_hm1", [NPAD, D], BF16)
    src_hT = din("src_hT", [C.in_dim, NPAD], BF16)
    wfc = din("wfc", [C.in_dim, D], BF16)
    m1 = din("m1", [C.in_dim, H], BF16)
    vmat = din("v", [C.ed, H], BF16)
    w1 = din("w1", [D, C.fh], BF16)
    w2 = din("w2", [C.fh, D], BF16)
    b1c = din("b1c", [C.fh, 1], F32)
    b2c = din("b2c", [D, 1], F32)
    g_rep = din("g_rep", [128, D], F32)
    b_rep = din("b_rep", [128, D], F32)
    identb = din("identb", [128, 128], BF16)

    out_shard = nc.dram_tensor("out_shard", [NPAD, D], F32,
                               kind="ExternalOutput").ap()

    zc_bounce = nc.dram_tensor("zc_bounce", [C.tgt_per, ZR], BF16).ap()
    zc_space = "Shared" if C.ncores > 4 else None
    zc_table = nc.dram_tensor("zc_table", [C.n_src, ZR], BF16,
                              addr_space=zc_space).ap()

    KT = C.in_dim // 128   # 4
    FT = D // 128          # 4
    MT1 = C.fh // 128      # 16
    # 512-col chunk while gathers pace the loop, then 256-col chunks so the
    # post-loop FFN tail is only one 256-col chunk + two LayerNorms
    chunks = [(0, 512), (512, 256), (768, 256), (1024, 256)]

    with tile.TileContext(nc) as tc, ExitStack() as top:
        const = top.enter_context(tc.tile_pool(name="const", bufs=1))
        zgp = top.enter_context(tc.tile_pool(name="zg", bufs=BUFS))

        # idx table first: SWDGE preps consume it
        idx_sb = const.tile([128, NBLK, CPB * 8], I16)
        nc.sync.dma_start(idx_sb[:], idxw[:, :, :].rearrange("b p s -> p b s"))
        v_sb = const.tile([128, H], BF16)
        nc.sync.dma_start(v_sb[:], vmat[:, :])
        idb_sb = const.tile([128, 128], BF16)
        nc.sync.dma_start(idb_sb[:], identb[:, :])
        zero_sb = const.tile([128, 1], F32)
        nc.vector.memset(zero_sb[:], 0.0)
        eps_sb = const.tile([128, 1], F32)
        nc.vector.memset(eps_sb[:], LN_EPS)
        zpad_sb = const.tile([128, ZR - D - H], BF16)
        nc.vector.memset(zpad_sb[:], 0.0)
        # (w1/w2/ln consts DMA'd after phase 0 below; allocated now)
        w1_sb = const.tile([128, KT, MT1, 128], BF16)
        w2_sb = const.tile([128, MT1, FT, 128], BF16)
        b1_sb = const.tile([128, MT1, 1], F32)
        b2_sb = const.tile([128, FT, 1], F32)
        grep_sb = const.tile([128, D], F32)
        brep_sb = const.tile([128, D], F32)

        zg_tiles = {}

        def emit_gather(nb):
            """Gather issued BUFS blocks ahead of consumption so GpSimd
            desc-gen overlaps compute on the other engines."""
            zg = zgp.tile([128, CPB, ZR], BF16, tag="zg")
            zg_tiles[nb] = zg
            for g0 in range(0, CPB, GCALL):
                gn = min(GCALL, CPB - g0)
                nc.gpsimd.dma_gather(
                    out_ap=zg[:, g0:g0 + gn, :], in_ap=zc_table[:, :],
                    idxs_ap=idx_sb[:, nb, g0 * 8:(g0 + gn) * 8],
                    num_idxs=gn * 128, num_idxs_reg=gn * 128, elem_size=ZR)

        # ---------------- phase 0: z rows + s1 rows -> zc_bounce -> AllGather
        with ExitStack() as p0:
            ps0 = p0.enter_context(tc.tile_pool(name="ps0", bufs=2, space="PSUM"))
            zr_pool = p0.enter_context(tc.tile_pool(name="zrow", bufs=2))
            shp = p0.enter_context(tc.tile_pool(name="shp", bufs=1))
            sh_sb = shp.tile([128, KT, NPAD], BF16)
            nc.sync.dma_start(sh_sb[:],
                              src_hT.rearrange("(kt p) n -> p kt n", p=128))
            wfc_sb = shp.tile([128, KT, D], BF16)
            nc.sync.dma_start(wfc_sb[:], wfc.rearrange("(kt p) m -> p kt m", p=128))
            m1_sb = shp.tile([128, KT, H], BF16)
            nc.sync.dma_start(m1_sb[:], m1.rearrange("(kt p) m -> p kt m", p=128))
            for nb in range(NBLK):
                rows = min(128, C.tgt_per - nb * 128)
                if rows <= 0:
                    break
                z_ps = ps0.tile([128, D], F32, tag="zps")
                for kt in range(KT):
                    nc.tensor.matmul(z_ps[:], sh_sb[:, kt, nb * 128:(nb + 1) * 128],
                                     wfc_sb[:, kt, :], start=(kt == 0),
                                     stop=(kt == KT - 1))
                s1_ps = ps0.tile([128, H], F32, tag="s1ps")
                for kt in range(KT):
                    nc.tensor.matmul(s1_ps[:], sh_sb[:, kt, nb * 128:(nb + 1) * 128],
                                     m1_sb[:, kt, :], start=(kt == 0),
                                     stop=(kt == KT - 1))
                zrow = zr_pool.tile([128, D], BF16, tag="zrow")
                nc.vector.tensor_copy(zrow[:], z_ps[:])
                s1row = zr_pool.tile([128, H], BF16, tag="s1row")
                nc.vector.tensor_copy(s1row[:], s1_ps[:])
                nc.sync.dma_start(zc_bounce[nb * 128:nb * 128 + rows, 0:D],
                                  zrow[0:rows, :])
                nc.sync.dma_start(zc_bounce[nb * 128:nb * 128 + rows, D:D + H],
                                  s1row[0:rows, :])
                nc.sync.dma_start(zc_bounce[nb * 128:nb * 128 + rows, D + H:ZR],
                                  zpad_sb[0:rows, :])

        nc.gpsimd.collective_compute(
            "AllGather", mybir.AluOpType.bypass,
            replica_groups=[list(range(C.ncores))],
            ins=[zc_bounce[:, :]], outs=[zc_table[:, :]],
        )

        for nb0 in range(BUFS):
            emit_gather(nb0)

        # FFN / LN constants (not needed until the first FFN chunk)
        nc.sync.dma_start(w1_sb[:], w1.rearrange("(kt p) (mt m) -> p kt mt m",
                                                 p=128, m=128))
        nc.sync.dma_start(w2_sb[:], w2.rearrange("(kt p) (mt m) -> p kt mt m",
                                                 p=128, m=128))
        nc.sync.dma_start(b1_sb[:], b1c.rearrange("(mt p) x -> p mt x", p=128))
        nc.sync.dma_start(b2_sb[:], b2c.rearrange("(mt p) x -> p mt x", p=128))
        nc.sync.dma_start(grep_sb[:], g_rep[:, :])
        nc.sync.dma_start(brep_sb[:], b_rep[:, :])

        # ---------------- blocks + FFN, interleaved so PE fills gather gaps
        with ExitStack() as pb:
            ps_s2 = pb.enter_context(tc.tile_pool(name="ps_s2", bufs=1, space="PSUM"))
            ps_hag = pb.enter_context(tc.tile_pool(name="ps_hag", bufs=1, space="PSUM"))
            ps_tp = pb.enter_context(tc.tile_pool(name="ps_tp", bufs=1, space="PSUM"))
            ps_a1 = pb.enter_context(tc.tile_pool(name="ps_a1", bufs=2, space="PSUM"))
            ps_o2 = pb.enter_context(tc.tile_pool(name="ps_o2", bufs=2, space="PSUM"))
            gpool = pb.enter_context(tc.tile_pool(name="ee_oh", bufs=2))
            epool = pb.enter_context(tc.tile_pool(name="escore", bufs=2))
            hpool = pb.enter_context(tc.tile_pool(name="hb", bufs=1))
            tgtp = pb.enter_context(tc.tile_pool(name="tgtp", bufs=2))
            hbtp = pb.enter_context(tc.tile_pool(name="hbt", bufs=1))
            fpool = pb.enter_context(tc.tile_pool(name="ffn", bufs=1))
            r1p = pb.enter_context(tc.tile_pool(name="r1", bufs=1))
            tmpp = pb.enter_context(tc.tile_pool(name="tmp", bufs=2))
            lnp = pb.enter_context(tc.tile_pool(name="ln", bufs=1))
            stp = pb.enter_context(tc.tile_pool(name="stat", bufs=2))

            hbT = hbtp.tile([128, FT, NPAD], BF16)
            r2 = fpool.tile([128, FT, NPAD], BF16)

            def emit_ffn_chunk(cs, cw):
                r1 = r1p.tile([128, MT1, cw], BF16, tag="r1")
                for mt in range(MT1):
                    a1 = ps_a1.tile([128, cw], F32, tag="a1")
                    for kt in range(KT):
                        nc.tensor.matmul(a1[:], w1_sb[:, kt, mt, :],
                                         hbT[:, kt, cs:cs + cw],
                                         start=(kt == 0), stop=(kt == KT - 1))
                    if mt % 2 == 0:
                        nc.scalar.activation(r1[:, mt, :], a1[:],
                                             mybir.ActivationFunctionType.Relu,
                                             bias=b1_sb[:, mt, :])
                    else:
                        nc.vector.tensor_scalar(r1[:, mt, :], a1[:],
                                                b1_sb[:, mt, :], 0.0,
                                                mybir.AluOpType.add,
                                                mybir.AluOpType.max)
                for mt2 in range(FT):
                    o2 = ps_o2.tile([128, cw], F32, tag="o2")
                    for kt2 in range(MT1):
                        nc.tensor.matmul(o2[:], w2_sb[:, kt2, mt2, :],
                                         r1[:, kt2, :],
                                         start=(kt2 == 0), stop=(kt2 == MT1 - 1))
                    t1 = tmpp.tile([128, cw], F32, tag="t1")
                    nc.vector.tensor_scalar_add(t1[:], o2[:], b2_sb[:, mt2, :])
                    nc.vector.tensor_tensor(r2[:, mt2, cs:cs + cw], t1[:],
                                            hbT[:, mt2, cs:cs + cw],
                                            mybir.AluOpType.add)

            def emit_ln(nb):
                r2n = lnp.tile([128, D], BF16, tag="r2n")
                for ft in range(FT):
                    tp = ps_tp.tile([128, 128], BF16, tag="tp")
                    nc.tensor.transpose(tp[:], r2[:, ft, nb * 128:(nb + 1) * 128],
                                        idb_sb[:])
                    nc.vector.tensor_copy(r2n[:, ft * 128:(ft + 1) * 128], tp[:])
                scrap = lnp.tile([128, D], BF16, tag="scrap")
                ssum = stp.tile([128, 1], F32, tag="ssum")
                nc.scalar.activation(scrap[:], r2n[:],
                                     mybir.ActivationFunctionType.Copy,
                                     accum_out=ssum[:])
                mu = stp.tile([128, 1], F32, tag="mu")
                nc.vector.tensor_scalar_mul(mu[:], ssum[:], 1.0 / D)
                xc = lnp.tile([128, D], BF16, tag="xc")
                nc.vector.tensor_scalar(xc[:], r2n[:], mu[:], None,
                                        mybir.AluOpType.subtract)
                sq = lnp.tile([128, D], BF16, tag="sq")
                ssq = stp.tile([128, 1], F32, tag="ssq")
                nc.scalar.activation(sq[:], xc[:],
                                     mybir.ActivationFunctionType.Square,
                                     accum_out=ssq[:])
                std = stp.tile([128, 1], F32, tag="std")
                nc.scalar.activation(std[:], ssq[:],
                                     mybir.ActivationFunctionType.Sqrt,
                                     bias=eps_sb[:, :], scale=1.0 / D)
                rstd = stp.tile([128, 1], F32, tag="rstd")
                nc.vector.reciprocal(rstd[:], std[:])
                xn = lnp.tile([128, D], BF16, tag="xn")
                nc.vector.tensor_scalar_mul(xn[:], xc[:], rstd[:])
                xg = lnp.tile([128, D], F32, tag="xg")
                nc.vector.tensor_tensor(xg[:], xn[:], grep_sb[:],
                                        mybir.AluOpType.mult)
                orow = lnp.tile([128, D], F32, tag="orow")
                nc.vector.tensor_tensor(orow[:], xg[:], brep_sb[:],
                                        mybir.AluOpType.add)
                nc.sync.dma_start(out_shard[nb * 128:(nb + 1) * 128, :], orow[:])

            next_chunk = 0
            for nb in range(NBLK):
                zg = zg_tiles[nb]
                ee_t = gpool.tile([128, CPB, 128], BF16, tag="ee")
                nc.sync.dma_start(ee_t[:], eeT[nb].rearrange("c d e -> d c e"))
                oh_t = gpool.tile([128, CPB, 128], BF16, tag="oh")
                nc.sync.dma_start(oh_t[:], oh[nb].rearrange("c e t -> e c t"))
                tgtb = tgtp.tile([128, D], BF16, tag="tgtb")
                nc.sync.dma_start(tgtb[:], tgt_hm1[nb * 128:(nb + 1) * 128, :])

                s2_ps = ps_s2.tile([128, CPB * H], F32, tag="s2")
                for j in range(CPB):
                    nc.tensor.matmul(s2_ps[:, j * H:(j + 1) * H], ee_t[:, j, :],
                                     v_sb[:, :], start=True, stop=True)
                e1 = epool.tile([128, CPB, H], F32, tag="e1")
                nc.vector.tensor_tensor(
                    e1[:, :, :],
                    s2_ps[:, :].rearrange("p (c h) -> p c h", h=H),
                    zg[:, :, D:D + H],
                    mybir.AluOpType.add)
                lk = epool.tile([128, CPB, H], F32, tag="lk")
                nc.vector.tensor_scalar_mul(lk[:, :, :], e1[:, :, :], LEAK)
                e2 = epool.tile([128, CPB, H], F32, tag="e2")
                nc.vector.tensor_tensor(e2[:, :, :], e1[:, :, :], lk[:, :, :],
                                        mybir.AluOpType.max)
                eexp = epool.tile([128, CPB, H], BF16, tag="eexp")
                nc.scalar.activation(eexp[:, :, :], e2[:, :, :],
                                     mybir.ActivationFunctionType.Exp,
                                     bias=zero_sb[:, :])

                # eexp weighting in place into the gathered z columns
                nc.vector.tensor_tensor(
                    zg[:, :, 0:D].rearrange("p c (o h) -> p c o h", h=H),
                    zg[:, :, 0:D].rearrange("p c (o h) -> p c o h", h=H),
                    eexp[:, :, :].rearrange("p c (h x) -> p c x h", x=1)
                        .broadcast_to([128, CPB, O, H]),
                    mybir.AluOpType.mult)

                hag = ps_hag.tile([128, D + H], F32, tag="hag")
                for j in range(CPB):
                    nc.tensor.matmul(hag[:, 0:D], oh_t[:, j, :], zg[:, j, 0:D],
                                     start=(j == 0), stop=(j == CPB - 1),
                                     skip_group_check=True)
                    nc.tensor.matmul(hag[:, D:D + H], oh_t[:, j, :], eexp[:, j, :],
                                     start=(j == 0), stop=(j == CPB - 1),
                                     skip_group_check=True)

                den = epool.tile([128, H], F32, tag="den")
                nc.vector.tensor_scalar_max(den[:], hag[:, D:D + H], 1e-30)
                rec = epool.tile([128, H], F32, tag="rec")
                nc.vector.reciprocal(rec[:], den[:])

                hbp = hpool.tile([128, D], BF16, tag="hbp")
                nc.vector.tensor_tensor(
                    hbp[:, :].rearrange("p (h o) -> p h o", o=O),
                    hag[:, 0:D].rearrange("p (o h) -> p h o", h=H),
                    rec[:, :].rearrange("p (h x) -> p h x", x=1)
                        .broadcast_to([128, H, O]),
                    mybir.AluOpType.mult)
                # elu(x) + tgt = max(x,0) + min(exp(x),1) + (tgt-1)
                mx = hpool.tile([128, D], BF16, tag="mx")
                nc.scalar.activation(mx[:], hbp[:],
                                     mybir.ActivationFunctionType.Relu)
                ex = hpool.tile([128, D], BF16, tag="ex")
                nc.scalar.activation(ex[:], hbp[:],
                                     mybir.ActivationFunctionType.Exp,
                                     bias=zero_sb[:, :])
                ex1 = hpool.tile([128, D], BF16, tag="ex1")
                nc.vector.tensor_scalar_min(ex1[:], ex[:], 1.0)
                hb2 = hpool.tile([128, D], BF16, tag="hb2")
                nc.vector.tensor_tensor(hb2[:], ex1[:], mx[:], mybir.AluOpType.add)
                hb3 = hpool.tile([128, D], BF16, tag="hb3")
                nc.vector.tensor_tensor(hb3[:], hb2[:], tgtb[:],
                                        mybir.AluOpType.add)
                for ft in range(FT):
                    tpb = ps_tp.tile([128, 128], BF16, tag="tp")
                    nc.tensor.transpose(tpb[:], hb3[:, ft * 128:(ft + 1) * 128],
                                        idb_sb[:])
                    nc.vector.tensor_copy(hbT[:, ft, nb * 128:(nb + 1) * 128],
                                          tpb[:])

                # prep+fire gather nb+BUFS (its zg slot was freed by block nb)
                if nb + BUFS < NBLK:
                    emit_gather(nb + BUFS)

                while (next_chunk < len(chunks)
                       and chunks[next_chunk][0] + chunks[next_chunk][1]
                       <= (nb + 1) * 128):
                    cs, cw = chunks[next_chunk]
                    emit_ffn_chunk(cs, cw)
                    for nb_ln in range(cs // 128, (cs + cw) // 128):
                        emit_ln(nb_ln)
                    next_chunk += 1

    nc.compile()
    return nc


_CACHE = {}


def _get_program(C):
    key = (C.ncores, C.n_src, C.n_tgt, C.e, C.cpb)
    if key not in _CACHE:
        _CACHE[key] = build_program(C)
    return _CACHE[key]


def kernel(src_h, tgt_h, edge_embed, edge_src, edge_dst,
           W_fc, W_feat, attn_a, w1, b1, w2, b2, ln_g, ln_b):
    from concourse.bass_utils import run_bass_kernel_spmd

    C = full_cfg()
    cores, shared, row_of_node = host_prep(
        C, src_h, tgt_h, edge_embed, edge_src, edge_dst,
        W_fc, W_feat, attn_a, w1, b1, w2, b2, ln_g, ln_b)
    nc = _get_program(C)
    in_maps = []
    for c in range(C.ncores):
        m = dict(shared)
        cc = cores[c]
        m.update(idxw=cc["idxw"], eeT=cc["eeT"], oh=cc["oh"],
                 tgt_hm1=cc["tgt_hm1"], src_hT=cc["src_hT"])
        in_maps.append(m)
    import os
    try:
        res = run_bass_kernel_spmd(nc, in_maps, list(range(C.ncores)))
    except Exception:
        if os.environ.get("BASS_TRACE"):
            os.environ["BASS_NEVER_TRACE"] = "1"
            res = run_bass_kernel_spmd(nc, in_maps, list(range(C.ncores)))
        else:
            raise
    global _last_results
    _last_results = res
    allrows = np.concatenate(
        [res.results[c]["out_shard"] for c in range(C.ncores)], axis=0)
    out = allrows[row_of_node]
    return np.ascontiguousarray(out, dtype=np.float32)
